# revision 1
# baseline (speedup 1.0000x reference)
"""DeBERTa-style 12-layer transformer on 8 TRN2 NeuronCores.

Sharding: data-parallel over batch (B=8 -> 1 sequence per core, no
collectives). Weights are host-prepped (transposed/blobbed/fp16) and
replicated per core. Relative-position tables are expanded on host into
per-layer T1/T2 tables; the (q,k)-dependent gather is done on device via
matmul + a strided "skew" DMA read from a DRAM scratch buffer, and the
positional terms are accumulated straight into the score PSUM by TensorE
(c2p via transpose-accumulate, p2c via identity-lhsT accumulate).

ScalarE uses only the exp_and_others ACT table set (Exp, Tanh, Identity,
Copy): LayerNorm rstd is a DVE quake-rsqrt, both gelus use the tanh
decomposition. The attention tail stays transposed ([feature, token]):
per-head-pair ctx^T accumulates in one PSUM bank, the gate projection is
emitted transposed, and the gate/softmax-denominator/LayerNorm of ctx*g
are folded into the out-projection epilogue via ones-matmul statistics.
The FFN is likewise transposed so W2 consumes the GeGLU output directly
(no un-transposes); its LayerNorm is folded into the W2 epilogue.
"""

import math
import numpy as np
import ml_dtypes

import concourse.bacc as bacc
import concourse.bass as bass
import concourse.mybir as mybir
from concourse import tile
from concourse.bass_utils import run_bass_kernel_spmd
from concourse.masks import make_identity

BF = ml_dtypes.bfloat16
F16 = np.float16
bf16 = mybir.dt.bfloat16
fp16 = mybir.dt.float16
f32 = mybir.dt.float32

V = 32768; H = 768; NH = 12; D = 64; L = 12; FI = 2048
S = 512; B = 8; BK = 32; MAXP = 512; EPS = 1e-7
SCALE = 1.0 / math.sqrt(3 * D)
NQT = S // 128      # 4 token tiles
NHT = H // 128      # 6 hidden tiles
WEXP = 640          # C-block width (per-tile expansion window)
MASK_NEG = -60000.0
G0 = 0.7978845608028654      # sqrt(2/pi)
G1 = 0.044715 * G0           # tanh-gelu cubic coefficient
RSQRT_MAGIC = 0x5F3759DF
MULT = mybir.AluOpType.mult
ADD = mybir.AluOpType.add
SUB = mybir.AluOpType.subtract


# ---------------------------------------------------------------- host math
def _beta_delta():
    """bucket(delta)+31 for delta in [-511, 511], indexed by delta+511."""
    delta = np.arange(-(S - 1), S)
    sign = np.sign(delta)
    mid = BK // 2
    abs_pos = np.where((delta < mid) & (delta > -mid), mid - 1,
                       np.minimum(np.abs(delta), MAXP - 1))
    log_pos = np.ceil(np.log(abs_pos / mid) / math.log((MAXP - 1) / mid)
                      * (mid - 1)).astype(np.int64) + mid
    bucket = np.where(abs_pos <= mid, delta, log_pos * sign).astype(np.int64)
    return bucket + BK - 1


def _ln_np(x):
    m = x.mean(-1, keepdims=True)
    v = x.var(-1, keepdims=True)
    return (x - m) / np.sqrt(v + EPS)


DBG = False


# ---------------------------------------------------------------- builder
def _build(n_layers):
    nc = bacc.Bacc("TRN2", target_bir_lowering=False, num_devices=B)
    dbg_g = dbg_cg = dbg_x = None
    if DBG:
        dbg_g = nc.dram_tensor("dbg_g", [6, 128, 512], fp16, kind="ExternalOutput")
        dbg_cg = nc.dram_tensor("dbg_cg", [6, 128, 512], fp16, kind="ExternalOutput")
        dbg_x = nc.dram_tensor("dbg_x", [NQT, 128, H], f32, kind="ExternalOutput")

    # ---- dram inputs (host-prepped layouts; partition-major weight blobs) ----
    wqk = nc.dram_tensor("wqk", [n_layers, 18, 128, 768], fp16, kind="ExternalInput")
    wvg = nc.dram_tensor("wvg", [n_layers, 128, 4608], fp16, kind="ExternalInput")
    wout = nc.dram_tensor("wout", [n_layers, 128, 4608], fp16, kind="ExternalInput")
    w1 = nc.dram_tensor("w1", [n_layers, 8, 128, 3072], fp16, kind="ExternalInput")
    w2 = nc.dram_tensor("w2", [n_layers, 2, 128, 6144], fp16, kind="ExternalInput")
    t1d = nc.dram_tensor("t1d", [n_layers, NH // 2, 128, 1024], fp16, kind="ExternalInput")
    t2d = nc.dram_tensor("t2d", [n_layers, NH // 2, 128, 1024], fp16, kind="ExternalInput")
    bqkd = nc.dram_tensor("bqkd", [n_layers, 128, 12], f32, kind="ExternalInput")
    bgd = nc.dram_tensor("bgd", [n_layers, 128, 6], f32, kind="ExternalInput")
    bvgd = nc.dram_tensor("bvgd", [n_layers, 1, H], fp16, kind="ExternalInput")
    boutd = nc.dram_tensor("boutd", [n_layers, 1, H], fp16, kind="ExternalInput")
    wsumd = nc.dram_tensor("wsumd", [n_layers, 1, 1536], fp16, kind="ExternalInput")
    x0d = nc.dram_tensor("x0d", [NQT, 128, H], f32, kind="ExternalInput")
    maskd = nc.dram_tensor("maskd", [128, NQT], f32, kind="ExternalInput")
    yd = nc.dram_tensor("yd", [NQT, 128, H], f32, kind="ExternalOutput")

    # dram scratch for positional C blocks (per layer, reused)
    c1d = nc.dram_tensor("c1d", [2, NH, NQT, 128, WEXP], fp16, kind="Internal")
    c2d = nc.dram_tensor("c2d", [2, NH, NQT, 128, WEXP], fp16, kind="Internal")
    CBLK = 128 * WEXP

    with tile.TileContext(nc) as tc:
        import contextlib
        ctx = contextlib.ExitStack()
        with ctx:
            pp = ctx.enter_context(tc.tile_pool(name="persist", bufs=1))
            sb = ctx.enter_context(tc.tile_pool(name="work", bufs=2))
            sb3 = ctx.enter_context(tc.tile_pool(name="work3", bufs=3))
            wpool = ctx.enter_context(tc.tile_pool(name="wts", bufs=4))
            ps_mm = ctx.enter_context(tc.tile_pool(name="psmm", bufs=3, space="PSUM"))
            ps_aux = ctx.enter_context(tc.tile_pool(name="psaux", bufs=1, space="PSUM"))
            ps_ctx = ctx.enter_context(tc.tile_pool(name="psctx", bufs=2, space="PSUM"))

            # persistent tiles
            x = [pp.tile([128, H], f32, name=f"x{qt}") for qt in range(NQT)]
            ident = pp.tile([128, 128], fp16, name="ident")
            make_identity(nc, ident[:])
            ones_col = pp.tile([128, 1], bf16, name="ones_col")
            nc.gpsimd.memset(ones_col[:], 1.0)
            ones_col16 = pp.tile([128, 1], fp16, name="ones_col16")
            nc.gpsimd.memset(ones_col16[:], 1.0)
            ones_row = pp.tile([1, 128], fp16, name="ones_row")
            nc.gpsimd.memset(ones_row[:], 1.0)
            ones_row32 = pp.tile([1, 64], f32, name="ones_row32")
            nc.gpsimd.memset(ones_row32[:], 1.0)
            one_f32 = pp.tile([1, 1], f32, name="one_f32")
            nc.gpsimd.memset(one_f32[:], 1.0)
            maskb = pp.tile([128, NQT], f32, name="maskb")
            nc.sync.dma_start(maskb[:], maskd[:])
            for qt in range(NQT):
                nc.sync.dma_start(x[qt][:], x0d[qt, :, :])

            # ---------------- helpers ----------------
            i32 = mybir.dt.int32

            def rsqrt_dve(dst, src, n, tagp):
                """dst[128, n] f32 = 1/sqrt(src[128, n]); DVE only (no ACT
                table). Quake seed + 2 Newton iterations (~4e-6 rel err)."""
                nc.vector.tensor_scalar(
                    dst[:].bitcast(i32), src[:].bitcast(i32), 1, None,
                    op0=mybir.AluOpType.logical_shift_right)
                nc.vector.tensor_scalar(
                    dst[:].bitcast(i32), dst[:].bitcast(i32), -1, RSQRT_MAGIC,
                    op0=MULT, op1=ADD)
                tmp = sb.tile([128, n], f32, tag=f"{tagp}rt")
                for _ in range(1):
                    nc.vector.tensor_tensor(tmp[:], dst[:], dst[:], op=MULT)
                    nc.vector.tensor_tensor(tmp[:], tmp[:], src[:], op=MULT)
                    nc.vector.tensor_scalar(tmp[:], tmp[:], -0.5, 1.5,
                                            op0=MULT, op1=ADD)
                    nc.vector.tensor_tensor(dst[:], dst[:], tmp[:], op=MULT)

            def ln_group(tiles, width, tagp, out_dtype=fp16, inplace=False):
                """Batched LayerNorm over len(tiles) tiles [128, width].
                Stats on DVE (incl. rsqrt), apply on ACT (Identity; no table
                switch). Returns normalized fp16 tiles."""
                n = len(tiles)
                mv = sb.tile([128, 2 * n], f32, tag=f"{tagp}mv")
                for i, t in enumerate(tiles):
                    if width == H:
                        chunks = [t[:, 0:384], t[:, 384:768]]
                    else:
                        chunks = [t[:, c * 512:(c + 1) * 512]
                                  for c in range(width // 512)]
                    stats = sb.tile([128, 6 * len(chunks)], f32, tag=f"{tagp}st")
                    for j, cap in enumerate(chunks):
                        nc.vector.bn_stats(stats[:, j * 6:(j + 1) * 6], cap)
                    nc.vector.bn_aggr(mv[:, 2 * i:2 * i + 2], stats[:])
                veps = sb.tile([128, n], f32, tag=f"{tagp}ve")
                for i in range(n):
                    nc.vector.tensor_scalar_add(veps[:, i:i + 1],
                                                mv[:, 2 * i + 1:2 * i + 2], EPS)
                rstd = sb.tile([128, n], f32, tag=f"{tagp}rs")
                rsqrt_dve(rstd, veps, n, tagp)
                outs = []
                for i, t in enumerate(tiles):
                    negb = sb.tile([128, 1], f32, tag=f"{tagp}nb")
                    nc.vector.scalar_tensor_tensor(
                        negb[:], mv[:, 2 * i:2 * i + 1], -1.0, rstd[:, i:i + 1],
                        op0=MULT, op1=MULT)
                    if inplace:
                        o = t
                    else:
                        o = sb.tile([128, width], out_dtype, tag=f"{tagp}{i}",
                                    name=f"{tagp}{i}", bufs=1)
                    nc.scalar.activation(o[:], t[:],
                                         mybir.ActivationFunctionType.Identity,
                                         bias=negb[:], scale=rstd[:, i:i + 1])
                    outs.append(o)
                return outs

            def transpose_h(tiles_fp16, nh_tiles, tag):
                """[128,q tiles][*, nh_tiles*128 wide] -> nh_tiles x [128, 512]."""
                outs = []
                for hc in range(nh_tiles):
                    pt = ps_mm.tile([128, 512], f32, tag="mm")
                    ptv = pt[:, 0:256].bitcast(fp16)
                    for qt in range(NQT):
                        nc.tensor.transpose(ptv[:, qt * 128:(qt + 1) * 128],
                                            tiles_fp16[qt][:, hc * 128:(hc + 1) * 128],
                                            ident[:])
                    o = sb.tile([128, 512], fp16, tag=f"hT{hc}", name=f"{tag}{hc}", bufs=1)
                    nc.scalar.copy(o[:], ptv[:])
                    outs.append(o)
                return outs

            def stats_to_cols(sum_ps, sqs_ps, nfeat, tagp):
                """[1,512] PSUM sums/sumsqs -> per-q-tile [128,4] columns of
                rstd*mean and -rstd (for folding LN into a matmul epilogue)."""
                srow = sb.tile([1, 1024], f32, tag=f"{tagp}sr")
                nc.vector.tensor_copy(srow[:, 0:512], sum_ps)
                nc.scalar.copy(srow[:, 512:1024], sqs_ps)
                stT_t = ps_aux.tile([128, 128], f32, tag="aux")
                stT = stT_t[:, 0:8]
                for qt in range(NQT):
                    nc.tensor.transpose(stT[:, qt:qt + 1],
                                        srow[:, qt * 128:(qt + 1) * 128],
                                        one_f32[:])
                    nc.tensor.transpose(stT[:, 4 + qt:5 + qt],
                                        srow[:, 512 + qt * 128:512 + (qt + 1) * 128],
                                        one_f32[:])
                st = sb.tile([128, 8], f32, tag=f"{tagp}stc")
                nc.vector.tensor_copy(st[:], stT)
                mean = sb.tile([128, 4], f32, tag=f"{tagp}mn")
                nc.vector.tensor_scalar(mean[:], st[:, 0:4], 1.0 / nfeat, None,
                                        op0=MULT)
                var = sb.tile([128, 4], f32, tag=f"{tagp}vr")
                nc.vector.tensor_tensor(var[:], mean[:], mean[:], op=MULT)
                nc.vector.scalar_tensor_tensor(var[:], st[:, 4:8], 1.0 / nfeat,
                                               var[:], op0=MULT, op1=SUB)
                nc.vector.tensor_scalar_add(var[:], var[:], EPS)
                rstd = sb.tile([128, 4], f32, tag=f"{tagp}rsd")
                rsqrt_dve(rstd, var, 4, tagp)
                negrstd = sb.tile([128, 4], f32, tag=f"{tagp}nr")
                nc.vector.tensor_scalar(negrstd[:], rstd[:], -1.0, None, op0=MULT)
                return mean, negrstd

            def gelu_tanh_inplace(gg, tagp):
                """gg <- gelu(gg) (tanh approx); polynomial on GpSimd, Tanh on
                ACT. gg is a [128, 512] fp16 tile."""
                gsq = sb.tile([128, 512], fp16, tag=f"{tagp}sq")
                nc.gpsimd.tensor_tensor(gsq[:], gg[:], gg[:], op=MULT)
                nc.gpsimd.tensor_scalar(gsq[:], gsq[:], G1, G0, op0=MULT, op1=ADD)
                nc.gpsimd.tensor_tensor(gsq[:], gsq[:], gg[:], op=MULT)
                th = sb.tile([128, 512], fp16, tag=f"{tagp}th")
                nc.scalar.activation(th[:], gsq[:],
                                     mybir.ActivationFunctionType.Tanh)
                nc.gpsimd.tensor_scalar(th[:], th[:], 0.5, 0.5, op0=MULT, op1=ADD)
                nc.gpsimd.tensor_tensor(gg[:], gg[:], th[:], op=MULT)

            # ---------------- layers ----------------
            for li in range(n_layers):
                par = li % 2
                # ---- attention input LN + transpose ----
                hs = ln_group(x, H, "hs")                   # 4 x [128,768] fp16
                hsT = transpose_h(hs, NHT, "hsT")           # 6 x [128,512] fp16

                # ---- QK^T + gate projections: 18 o-tiles [128, 512] ----
                qkT = []
                gT = []
                bqk_sb = sb.tile([128, 12], f32, tag="bqk")
                nc.sync.dma_start(bqk_sb[:], bqkd[li, :, :])
                bg_sb = sb.tile([128, 6], f32, tag="bg")
                nc.sync.dma_start(bg_sb[:], bgd[li, :, :])
                for ot in range(18):
                    wq = wpool.tile([128, 768], fp16, tag="wqk", bufs=2)
                    nc.gpsimd.dma_start(wq[:], wqk[li, ot, :, :])
                    po = ps_mm.tile([128, 512], f32, tag="mm")
                    for hc in range(NHT):
                        nc.tensor.matmul(po[:], wq[:, hc * 128:(hc + 1) * 128],
                                         hsT[hc][:],
                                         start=(hc == 0), stop=(hc == NHT - 1))
                    if ot < 12:
                        o = sb.tile([128, 512], fp16, tag=f"tp{ot}",
                                    name=f"qkT{ot}", bufs=1)
                        sc = SCALE if ot < 6 else 1.0
                        nc.scalar.activation(o[:], po[:],
                                             mybir.ActivationFunctionType.Identity,
                                             bias=bqk_sb[:, ot:ot + 1], scale=sc)
                        qkT.append(o)
                    else:
                        o = sb.tile([128, 512], fp16, tag=f"gT{ot - 12}",
                                    name=f"gT{ot - 12}", bufs=1)
                        nc.scalar.activation(o[:], po[:],
                                             mybir.ActivationFunctionType.Identity,
                                             bias=bg_sb[:, ot - 12:ot - 11])
                        gT.append(o)

                # ---- V projection: natural layout [tok, o] ----
                v_sb = [sb.tile([128, H], bf16, tag=f"v{tt}", name=f"v{tt}", bufs=1)
                        for tt in range(NQT)]
                wv = wpool.tile([128, 4608], fp16, tag="wvg", bufs=1)
                nc.gpsimd.dma_start(wv[:], wvg[li, :, :])
                bv = wpool.tile([1, H], fp16, tag="bvg", bufs=2)
                nc.sync.dma_start(bv[:], bvgd[li, :, :])
                for lo, w in ((0, 512), (512, 256)):
                    for tt in range(NQT):
                        po = ps_mm.tile([128, 512], f32, tag="mm")
                        for hc in range(NHT):
                            nc.tensor.matmul(po[:, :w],
                                             hsT[hc][:, tt * 128:(tt + 1) * 128],
                                             wv[:, hc * 768 + lo:hc * 768 + lo + w],
                                             start=(hc == 0), stop=False,
                                             skip_group_check=True)
                        nc.tensor.matmul(po[:, :w], ones_row[:], bv[:, lo:lo + w],
                                         start=False, stop=True,
                                         skip_group_check=True)
                        nc.scalar.copy(v_sb[tt][:, lo:lo + w], po[:, :w])

                # ---- positional C-block expansion pre-pass (all heads);
                # the writes overlap the V/G projections, so the per-head
                # skew reads below never wait on the DRAM round trip.
                for pr in range(NH // 2):
                    t1_sb = sb3.tile([128, 1024], fp16, tag="t1", bufs=2)
                    nc.sync.dma_start(t1_sb[:], t1d[li, pr, :, :])
                    t2_sb = sb3.tile([128, 1024], fp16, tag="t2", bufs=2)
                    nc.sync.dma_start(t2_sb[:], t2d[li, pr, :, :])
                    for sub in range(2):
                        h = 2 * pr + sub
                        hp = sub * 64
                        qT_h = qkT[pr][hp:hp + 64, :]
                        kT_h = qkT[6 + pr][hp:hp + 64, :]
                        for tsb, lhs_full, cdram in (
                                (t1_sb, qT_h, c1d), (t2_sb, kT_h, c2d)):
                            for bt in range(NQT):
                                j0 = 384 - 128 * bt
                                pa = ps_mm.tile([128, 512], f32, tag="mm")
                                nc.tensor.matmul(pa[:], lhs_full[:, bt * 128:(bt + 1) * 128],
                                                 tsb[hp:hp + 64, j0:j0 + 512],
                                                 start=True, stop=True)
                                pb_t = ps_aux.tile([128, 128], f32, tag="aux")
                                pb = pb_t[:, 0:128]
                                nc.tensor.matmul(pb, lhs_full[:, bt * 128:(bt + 1) * 128],
                                                 tsb[hp:hp + 64, j0 + 512:j0 + 640],
                                                 start=True, stop=True)
                                stg = sb3.tile([128, WEXP], fp16, tag="cstg",
                                               bufs=3)
                                nc.vector.tensor_copy(stg[:, 0:512], pa[:])
                                nc.scalar.copy(stg[:, 512:WEXP], pb)
                                eng = nc.sync if cdram is c1d else nc.gpsimd
                                eng.dma_start(cdram[par, h, bt, :, :], stg[:])

                # ---- attention, per head-pair; ctx kept transposed ----
                cgT = []        # 6 x [128, 512] fp16: ctx^T * gelu(g) / den
                sq_tiles = []
                for pr in range(NH // 2):
                    ctxP_ps = ps_ctx.tile([128, 512], f32, tag="ctxT")
                    recb_ps = ps_ctx.tile([128, 512], f32, tag="recb")
                    for sub in range(2):
                        h = 2 * pr + sub
                        hp = sub * 64
                        tpos = (0, hp) if hp else None
                        qT_h = qkT[pr][hp:hp + 64, :]
                        kT_h = qkT[6 + pr][hp:hp + 64, :]
                        # skew reads
                        cbase = ((par * NH + h) * NQT) * CBLK
                        c2p_sb = []
                        for qt in range(NQT):
                            t = sb3.tile([128, 512], fp16, tag=f"c2p{qt}",
                                         name=f"c2p{qt}", bufs=2)
                            ap = bass.AP(c1d, cbase + qt * CBLK + 127,
                                         [[WEXP - 1, 128], [1, 512]])
                            nc.sync.dma_start(t[:], ap)
                            c2p_sb.append(t[:])
                        p2c_sb = []
                        for kt in range(NQT):
                            t = sb3.tile([128, 512], fp16, tag="p2c", name="p2c",
                                         bufs=6)
                            ap = bass.AP(c2d, cbase + kt * CBLK + 127,
                                         [[WEXP - 1, 128], [1, 512]])
                            nc.gpsimd.dma_start(t[:], ap)
                            p2c_sb.append(t[:])
                        # scores: c2c + positional terms accumulate in PSUM;
                        # mask goes in as the Exp bias.
                        den_t = ps_ctx.tile([128, 512], f32, tag="recb")
                        den_ps = den_t[0:1, :]
                        for kt in range(NQT):
                            ps_s = ps_mm.tile([128, 512], f32, tag="mm")
                            nc.tensor.matmul(ps_s[:],
                                             kT_h[:, kt * 128:(kt + 1) * 128],
                                             qT_h[:], start=True, stop=True)
                            for qt in range(NQT):
                                nc.tensor.matmul(
                                    ps_s[:, qt * 128:(qt + 1) * 128],
                                    c2p_sb[qt][:, kt * 128:(kt + 1) * 128],
                                    ident[:], start=False, stop=True,
                                    skip_group_check=True)
                            nc.tensor.matmul(ps_s[:], ident[:], p2c_sb[kt],
                                             start=False, stop=True,
                                             skip_group_check=True)
                            pT = sb3.tile([128, 512], bf16, tag="pT", name="pT")
                            nc.scalar.activation(pT[:], ps_s[:],
                                                 mybir.ActivationFunctionType.Exp,
                                                 bias=maskb[:, kt:kt + 1])
                            nc.tensor.matmul(den_ps, ones_col[:], pT[:],
                                             start=(kt == 0), stop=(kt == NQT - 1),
                                             skip_group_check=True)
                            nc.tensor.matmul(ctxP_ps[hp:hp + 64, :],
                                             v_sb[kt][:, h * 64:(h + 1) * 64],
                                             pT[:],
                                             start=(kt == 0), stop=(kt == NQT - 1),
                                             skip_group_check=True,
                                             tile_position=tpos)
                        rec = sb.tile([1, 512], f32, tag="rec")
                        nc.vector.reciprocal_approx_fast(rec[:], den_ps)
                        nc.tensor.matmul(recb_ps[hp:hp + 64, :], ones_row32[:],
                                         rec[:], start=True, stop=True,
                                         skip_group_check=True,
                                         tile_position=tpos)
                    # cg = ctx^T * gelu(g) * (1/den), all in [feature, token]
                    if DBG:
                        nc.sync.dma_start(dbg_g[pr, :, :], gT[pr][:])
                    gelu_tanh_inplace(gT[pr], "gl")
                    cg = sb.tile([128, 512], fp16, tag=f"cg{pr}", name=f"cg{pr}",
                                 bufs=1)
                    nc.vector.tensor_tensor(cg[:], ctxP_ps[:], gT[pr][:], op=MULT)
                    nc.vector.tensor_tensor(cg[:], cg[:], recb_ps[:], op=MULT)
                    if DBG:
                        nc.sync.dma_start(dbg_cg[pr, :, :], cg[:])
                    cgT.append(cg)
                    sq = sb.tile([128, 512], fp16, tag=f"gT{pr}", name=f"sq{pr}",
                                 bufs=1)
                    nc.gpsimd.tensor_tensor(sq[:], cg[:], cg[:], op=MULT)
                    sq_tiles.append(sq)

                # ---- cg LayerNorm stats (partition-axis, via ones-matmuls) ----
                sum_t = ps_ctx.tile([128, 512], f32, tag="recb")
                sqs_t = ps_ctx.tile([128, 512], f32, tag="recb")
                sum_ps, sqs_ps = sum_t[0:1, :], sqs_t[0:1, :]
                for i, cg in enumerate(cgT):
                    nc.tensor.matmul(sum_ps, ones_col16[:], cg[:],
                                     start=(i == 0), stop=(i == NHT - 1),
                                     skip_group_check=True)
                    nc.tensor.matmul(sqs_ps, ones_col16[:], sq_tiles[i][:],
                                     start=(i == 0), stop=(i == NHT - 1),
                                     skip_group_check=True)
                rmean_c, negrstd_c = stats_to_cols(sum_ps, sqs_ps, H, "cgs")

                # broadcast rows: ones (x) Woutsum, ones (x) bout
                wsum_sb = wpool.tile([1, 1536], fp16, tag="wsum", bufs=2)
                nc.sync.dma_start(wsum_sb[:], wsumd[li, :, :])
                bo = wpool.tile([1, H], fp16, tag="bout", bufs=2)
                nc.sync.dma_start(bo[:], boutd[li, :, :])
                wob_sb = sb.tile([128, H], fp16, tag="wob", bufs=1)
                bb_sb = sb.tile([128, H], fp16, tag="bb", bufs=1)
                for lo, w in ((0, 512), (512, 256)):
                    pw = ps_mm.tile([128, 512], f32, tag="mm")
                    nc.tensor.matmul(pw[:, :w], ones_row[:], wsum_sb[:, lo:lo + w],
                                     start=True, stop=True, skip_group_check=True)
                    nc.scalar.copy(wob_sb[:, lo:lo + w], pw[:, :w])
                    pb2 = ps_mm.tile([128, 512], f32, tag="mm")
                    nc.tensor.matmul(pb2[:, :w], ones_row[:], bo[:, lo:lo + w],
                                     start=True, stop=True, skip_group_check=True)
                    nc.scalar.copy(bb_sb[:, lo:lo + w], pb2[:, :w])

                # ---- out proj from cgT with LN folded into the epilogue ----
                wo = wpool.tile([128, 4608], fp16, tag="wout", bufs=1)
                nc.gpsimd.dma_start(wo[:], wout[li, :, :])
                for qt in range(NQT):
                    nc.vector.tensor_add(x[qt][:], x[qt][:], bb_sb[:])
                    veng = nc.vector
                    for lo, w in ((0, 512), (512, 256)):
                        po = ps_mm.tile([128, 512], f32, tag="mm")
                        for hc in range(NHT):
                            nc.tensor.matmul(po[:, :w],
                                             cgT[hc][:, qt * 128:(qt + 1) * 128],
                                             wo[:, hc * 768 + lo:hc * 768 + lo + w],
                                             start=(hc == 0), stop=(hc == NHT - 1),
                                             skip_group_check=True)
                        pe = sb.tile([128, 512], fp16, tag="poev")
                        nc.scalar.copy(pe[:, :w], po[:, :w])
                        t = sb.tile([128, 512], f32, tag="fold")
                        veng.scalar_tensor_tensor(
                            t[:, :w], wob_sb[:, lo:lo + w], rmean_c[:, qt:qt + 1],
                            pe[:, :w], op0=MULT, op1=SUB)
                        veng.scalar_tensor_tensor(
                            x[qt][:, lo:lo + w], t[:, :w], negrstd_c[:, qt:qt + 1],
                            x[qt][:, lo:lo + w], op0=MULT, op1=ADD)

                if DBG and li == 0:
                    for qt in range(NQT):
                        nc.sync.dma_start(dbg_x[qt, :, :], x[qt][:])
                # ---- FFN (transposed: W1 emits [feature, token]) ----
                h2 = ln_group(x, H, "hs")
                h2T = transpose_h(h2, NHT, "h2T")
                a_tiles = [sb.tile([128, 512], fp16, tag=f"tp{i}", name=f"a{i}",
                                   bufs=1) for i in range(16)]
                sum2_t = ps_ctx.tile([128, 512], f32, tag="recb")
                sqs2_t = ps_ctx.tile([128, 512], f32, tag="recb")
                sum2_ps, sqs2_ps = sum2_t[0:1, :], sqs2_t[0:1, :]
                for g8 in range(8):
                    wf = wpool.tile([128, 3072], fp16, tag="w1", bufs=2)
                    nc.sync.dma_start(wf[:], w1[li, g8, :, :])
                    for otl in range(4):
                        ot = g8 * 4 + otl
                        po = ps_mm.tile([128, 512], f32, tag="mm")
                        for hc in range(NHT):
                            nc.tensor.matmul(
                                po[:],
                                wf[:, otl * 768 + hc * 128:otl * 768 + (hc + 1) * 128],
                                h2T[hc][:],
                                start=(hc == 0), stop=(hc == NHT - 1))
                        if ot < 16:
                            nc.scalar.copy(a_tiles[ot][:], po[:])
                        else:
                            at = a_tiles[ot - 16]
                            gt = sb.tile([128, 512], fp16, tag="ffng")
                            nc.vector.tensor_copy(gt[:], po[:])
                            gelu_tanh_inplace(gt, "gl")
                            nc.vector.tensor_mul(at[:], at[:], gt[:])
                            # tile (ot-16) is final: accumulate its LN stats now
                            i = ot - 16
                            nc.tensor.matmul(sum2_ps, ones_col16[:], at[:],
                                             start=(i == 0), stop=(i == 15),
                                             skip_group_check=True)
                            sq = sb.tile([128, 512], fp16, tag="sq")
                            nc.gpsimd.tensor_tensor(sq[:], at[:], at[:], op=MULT)
                            nc.tensor.matmul(sqs2_ps, ones_col16[:], sq[:],
                                             start=(i == 0), stop=(i == 15),
                                             skip_group_check=True)

                # ---- W2 halves (loaded early so the DMA overlaps stats) ----
                w2h = []
                for hh in range(2):
                    wt2 = wpool.tile([128, 6144], fp16, tag=f"w2h{hh}", bufs=1)
                    nc.sync.dma_start(wt2[:], w2[li, hh, :, :])
                    w2h.append(wt2)

                rmean2, negrstd2 = stats_to_cols(sum2_ps, sqs2_ps, FI, "uns")
                w2b_sb = sb.tile([128, H], fp16, tag="w2b", bufs=1)
                for lo, w in ((0, 512), (512, 256)):
                    pw = ps_mm.tile([128, 512], f32, tag="mm")
                    nc.tensor.matmul(pw[:, :w], ones_row[:],
                                     wsum_sb[:, 768 + lo:768 + lo + w],
                                     start=True, stop=True, skip_group_check=True)
                    nc.scalar.copy(w2b_sb[:, lo:lo + w], pw[:, :w])

                # ---- W2 from raw GeGLU tiles with LN folded in ----
                for qt in range(NQT):
                    veng = nc.vector
                    for lo, w in ((0, 512), (512, 256)):
                        po = ps_mm.tile([128, 512], f32, tag="mm")
                        for ic in range(16):
                            nc.tensor.matmul(
                                po[:, :w], a_tiles[ic][:, qt * 128:(qt + 1) * 128],
                                w2h[ic // 8][:, (ic % 8) * 768 + lo:(ic % 8) * 768 + lo + w],
                                start=(ic == 0), stop=(ic == 15),
                                skip_group_check=True)
                        pe = sb.tile([128, 512], fp16, tag="poev")
                        nc.scalar.copy(pe[:, :w], po[:, :w])
                        t = sb.tile([128, 512], f32, tag="fold")
                        veng.scalar_tensor_tensor(
                            t[:, :w], w2b_sb[:, lo:lo + w], rmean2[:, qt:qt + 1],
                            pe[:, :w], op0=MULT, op1=SUB)
                        veng.scalar_tensor_tensor(
                            x[qt][:, lo:lo + w], t[:, :w], negrstd2[:, qt:qt + 1],
                            x[qt][:, lo:lo + w], op0=MULT, op1=ADD)

            # ---- output ----
            for qt in range(NQT):
                nc.sync.dma_start(yd[qt, :, :], x[qt][:])

    nc.finalize()
    return nc


_CACHE = {}


def _get_nc(n_layers):
    if n_layers not in _CACHE:
        _CACHE[n_layers] = _build(n_layers)
    return _CACHE[n_layers]


# ---------------------------------------------------------------- host prep
def _prep_shared(word_emb, rel_emb, rel_g, rel_b, Wqk, bqk, Wvg, bvg, Wout,
                 bout, W1, W2, n_layers):
    beta = _beta_delta()                     # [1023]
    idx_c2p = beta[1022 - np.arange(1023)]   # T1: delta = 511 - j
    idx_p2c = beta[np.arange(1023)]          # T2: delta = j - 511
    rel = _ln_np(rel_emb.astype(np.float64)).astype(np.float32) * rel_g + rel_b

    d = {}
    t1 = np.zeros((n_layers, NH, 64, 1024), np.float32)  # packed to pairs below
    t2 = np.zeros((n_layers, NH, 64, 1024), np.float32)
    wqk_t = np.zeros((n_layers, 18, 128, 768), np.float32)
    wvg_t = np.zeros((n_layers, 128, 4608), np.float32)
    wout_t = np.zeros((n_layers, 128, 4608), np.float32)
    w1_t = np.zeros((n_layers, 8, 128, 3072), np.float32)
    w2_t = np.zeros((n_layers, 2, 128, 6144), np.float32)
    bqk_t = np.zeros((n_layers, 128, 12), np.float32)
    bg_t = np.zeros((n_layers, 128, 6), np.float32)
    bvg_t = np.zeros((n_layers, 1, H), np.float32)
    bout_t = np.zeros((n_layers, 1, H), np.float32)
    wsum_t = np.zeros((n_layers, 1, 1536), np.float32)
    for li in range(n_layers):
        pos = rel @ Wqk[li].T + bqk[li]          # [63, 1536]
        qpos = pos[:, :H].reshape(63, NH, 64)
        kpos = pos[:, H:].reshape(63, NH, 64)
        # T1[j] = kpos[beta(511-j)], T2[j] = qpos[beta(j-511)] * SCALE
        t1[li, :, :, :1023] = kpos[idx_c2p].transpose(1, 2, 0)
        t2[li, :, :, :1023] = qpos[idx_p2c].transpose(1, 2, 0) * SCALE

        wqkT = Wqk[li].T.copy()                  # [768, 1536]
        # qk blobs: [ot, p, hc*128+oo] = wqkT[hc*128+p, ot*128+oo]
        wqk_t[li, :12] = (wqkT.reshape(NHT, 128, 12, 128)
                          .transpose(2, 1, 0, 3).reshape(12, 128, 768))
        wvgT = Wvg[li].T.copy()                  # [768, 1536]
        # gate blobs (transposed proj): [got, p, hc*128+oo]
        wqk_t[li, 12:] = (wvgT[:, H:].reshape(NHT, 128, NHT, 128)
                          .transpose(2, 1, 0, 3).reshape(6, 128, 768))
        # V rhs blob: [p, hc*768+c] = wvgT[hc*128+p, c]
        wvg_t[li] = (wvgT[:, :H].reshape(NHT, 128, H)
                     .transpose(1, 0, 2).reshape(128, 4608))
        wout_t[li] = (Wout[li].T.reshape(NHT, 128, H)
                      .transpose(1, 0, 2).reshape(128, 4608))
        # W1 lhsT blobs (transposed proj): [g, p, otl*768 + hc*128 + oo]
        w1_t[li] = (W1[li].T.reshape(NHT, 128, 32, 128)
                    .transpose(2, 1, 0, 3).reshape(8, 4, 128, NHT * 128)
                    .transpose(0, 2, 1, 3).reshape(8, 128, 3072))
        w2_t[li] = (W2[li].T.reshape(2, 8, 128, H)
                    .transpose(0, 2, 1, 3).reshape(2, 128, 6144))
        bqk_t[li] = bqk[li].reshape(12, 128).T
        bg_t[li] = bvg[li][H:].reshape(6, 128).T
        bvg_t[li, 0] = bvg[li][:H]
        bout_t[li, 0] = bout[li]
        wsum_t[li, 0, :H] = Wout[li].sum(axis=1)
        wsum_t[li, 0, H:] = W2[li].sum(axis=1)

    d["wqk"] = wqk_t.astype(F16)
    d["wvg"] = wvg_t.astype(F16)
    d["wout"] = wout_t.astype(F16)
    d["w1"] = w1_t.astype(F16)
    d["w2"] = w2_t.astype(F16)
    d["t1d"] = t1.reshape(n_layers, NH // 2, 128, 1024).astype(F16)
    d["t2d"] = t2.reshape(n_layers, NH // 2, 128, 1024).astype(F16)
    d["bqkd"] = bqk_t
    d["bgd"] = bg_t
    d["bvgd"] = bvg_t.astype(F16)
    d["boutd"] = bout_t.astype(F16)
    d["wsumd"] = wsum_t.astype(F16)
    return d


def _make_in_maps(inputs, n_layers):
    input_ids = np.asarray(inputs["input_ids"])
    attention_mask = np.asarray(inputs["attention_mask"])
    word_emb = np.asarray(inputs["word_emb"], np.float32)

    shared = _prep_shared(
        word_emb, np.asarray(inputs["rel_emb"], np.float32),
        np.asarray(inputs["rel_g"], np.float32), np.asarray(inputs["rel_b"], np.float32),
        np.asarray(inputs["Wqk"], np.float32), np.asarray(inputs["bqk"], np.float32),
        np.asarray(inputs["Wvg"], np.float32), np.asarray(inputs["bvg"], np.float32),
        np.asarray(inputs["Wout"], np.float32), np.asarray(inputs["bout"], np.float32),
        np.asarray(inputs["W1"], np.float32), np.asarray(inputs["W2"], np.float32),
        n_layers)

    in_maps = []
    for b in range(B):
        m = dict(shared)
        x0 = _ln_np(word_emb[input_ids[:, b]].astype(np.float64)).astype(np.float32)
        m["x0d"] = x0.reshape(NQT, 128, H)
        mb = np.where(attention_mask[b, 0, 0, :], MASK_NEG, 0.0).astype(np.float32)
        m["maskd"] = mb.reshape(NQT, 128).T.copy()
        in_maps.append(m)
    return in_maps


def run(inputs, n_layers=L, trace=False):
    nc = _get_nc(n_layers)
    in_maps = _make_in_maps(inputs, n_layers)
    res = run_bass_kernel_spmd(nc, in_maps, core_ids=list(range(B)), trace=trace)
    out = np.zeros((S, B, H), np.float32)
    for b in range(B):
        out[:, b, :] = res.results[b]["yd"].reshape(S, H)
    return out, res


def kernel(**inputs) -> np.ndarray:
    out, _ = run(inputs, L)
    return out


# ------------------------------------------------------- timing-only runner
def make_timed_runner(n_layers, in_maps):
    """Build a persistent jitted PJRT callable over 8 cores for wall-clock
    timing (the axon NTFF profile hook is unavailable in this container)."""
    import jax
    from jax.sharding import Mesh, PartitionSpec, NamedSharding
    from jax.experimental.shard_map import shard_map
    from concourse import bass2jax

    nc = _get_nc(n_layers)
    bass2jax.install_neuronx_cc_hook()
    partition_name = nc.partition_id_tensor.name if nc.partition_id_tensor else None
    in_names, out_names, out_avals, zero_outs = [], [], [], []
    import concourse.mybir as _mb
    for alloc in nc.m.functions[0].allocations:
        if not isinstance(alloc, _mb.MemoryLocationSet):
            continue
        name = alloc.memorylocations[0].name
        if alloc.kind == "ExternalInput":
            if name != partition_name:
                in_names.append(name)
        elif alloc.kind == "ExternalOutput":
            out_names.append(name)
            shape = tuple(alloc.tensor_shape)
            dtype = _mb.dt.np(alloc.dtype)
            out_avals.append(jax.core.ShapedArray(shape, dtype))
            zero_outs.append(np.zeros(shape, dtype))
    n_params = len(in_names)
    n_outs = len(out_avals)
    all_in_names = list(in_names) + out_names
    if partition_name is not None:
        all_in_names = all_in_names + [partition_name]

    def _body(*args):
        operands = list(args)
        if partition_name is not None:
            operands.append(bass2jax.partition_id_tensor())
        outs = bass2jax._bass_exec_p.bind(
            *operands, out_avals=tuple(out_avals), in_names=tuple(all_in_names),
            out_names=tuple(out_names), lowering_input_output_aliases=(),
            sim_require_finite=True, sim_require_nnan=True, nc=nc)
        return tuple(outs)

    n_cores = B
    devices = jax.devices()[:n_cores]
    mesh = Mesh(np.asarray(devices), ("core",))
    P = PartitionSpec
    sharded = jax.jit(
        shard_map(_body, mesh=mesh, in_specs=(P("core"),) * (n_params + n_outs),
                  out_specs=(P("core"),) * n_outs, check_rep=False),
        keep_unused=True)

    concat_in = [
        np.concatenate([np.asarray(in_maps[c][nm]) for c in range(n_cores)], axis=0)
        for nm in in_names]
    concat_zeros = [np.zeros((n_cores * z.shape[0], *z.shape[1:]), z.dtype)
                    for z in zero_outs]
    shard = NamedSharding(mesh, P("core"))
    dev_in = [jax.device_put(a, shard) for a in concat_in]
    dev_zeros = [jax.device_put(a, shard) for a in concat_zeros]

    def call():
        outs = sharded(*dev_in, *dev_zeros)
        jax.block_until_ready(outs)
        return outs

    return call



# revision 2
# speedup vs baseline: 20.6748x; 20.6748x over previous
"""DeBERTa-style 12-layer transformer on 8 TRN2 NeuronCores.

Sharding: data-parallel over batch (B=8 -> 1 sequence per core, no
collectives). Weights are host-prepped (transposed/blobbed/fp16) and
replicated per core. Relative-position tables are expanded on host into
per-layer T1/T2 tables; the (q,k)-dependent gather is done on device via
matmul + a strided "skew" DMA read from a DRAM scratch buffer, and the
positional terms are accumulated straight into the score PSUM by TensorE
(c2p via transpose-accumulate, p2c via identity-lhsT accumulate).

v2: all DMAs ride the two HWDGE rings (SP + ACT) so the Pool engine is
free; the positional C-block expansion emits fp16 directly into PSUM and
is staged with one copy + 2-block batched writes and 4-in-1 merged skew
reads; every gelu is a native ACT Gelu_apprx_tanh, batched so the ACT
table set switches only twice per layer (exp <-> gelu). The attention
tail stays transposed ([feature, token]): per-head-pair ctx^T accumulates
in one PSUM bank, the softmax denominator folds in via a DVE multiply
with the reciprocal broadcast, and the LayerNorms of ctx*g / GeGLU fold
into the out-projection / W2 epilogues via ones-matmul statistics.
"""

import math
import numpy as np
import ml_dtypes

import concourse.bacc as bacc
import concourse.bass as bass
import concourse.mybir as mybir
from concourse import tile
from concourse.bass_utils import run_bass_kernel_spmd
from concourse.masks import make_identity

BF = ml_dtypes.bfloat16
F16 = np.float16
bf16 = mybir.dt.bfloat16
fp16 = mybir.dt.float16
f32 = mybir.dt.float32

V = 32768; H = 768; NH = 12; D = 64; L = 12; FI = 2048
S = 512; B = 8; BK = 32; MAXP = 512; EPS = 1e-7
SCALE = 1.0 / math.sqrt(3 * D)
NQT = S // 128      # 4 token tiles
NHT = H // 128      # 6 hidden tiles
WEXP = 640          # C-block width (per-tile expansion window)
MASK_NEG = -60000.0
RSQRT_MAGIC = 0x5F3759DF
MULT = mybir.AluOpType.mult
ADD = mybir.AluOpType.add
SUB = mybir.AluOpType.subtract
GELU = mybir.ActivationFunctionType.Gelu_apprx_tanh
EXPF = mybir.ActivationFunctionType.Exp
IDENT = mybir.ActivationFunctionType.Identity


# ---------------------------------------------------------------- host math
def _beta_delta():
    """bucket(delta)+31 for delta in [-511, 511], indexed by delta+511."""
    delta = np.arange(-(S - 1), S)
    sign = np.sign(delta)
    mid = BK // 2
    abs_pos = np.where((delta < mid) & (delta > -mid), mid - 1,
                       np.minimum(np.abs(delta), MAXP - 1))
    log_pos = np.ceil(np.log(abs_pos / mid) / math.log((MAXP - 1) / mid)
                      * (mid - 1)).astype(np.int64) + mid
    bucket = np.where(abs_pos <= mid, delta, log_pos * sign).astype(np.int64)
    return bucket + BK - 1


def _ln_np(x):
    m = x.mean(-1, keepdims=True)
    v = x.var(-1, keepdims=True)
    return (x - m) / np.sqrt(v + EPS)


# ---------------------------------------------------------------- builder
def _build(n_layers, passes=1):
    nc = bacc.Bacc("TRN2", target_bir_lowering=False, num_devices=B)

    # ---- dram inputs (host-prepped layouts; partition-major weight blobs) ----
    wqk = nc.dram_tensor("wqk", [n_layers, 6, 128, 2304], fp16, kind="ExternalInput")
    wvg = nc.dram_tensor("wvg", [n_layers, 128, 4608], fp16, kind="ExternalInput")
    wout = nc.dram_tensor("wout", [n_layers, 128, 4608], fp16, kind="ExternalInput")
    w1 = nc.dram_tensor("w1", [n_layers, 8, 128, 3072], fp16, kind="ExternalInput")
    w2 = nc.dram_tensor("w2", [n_layers, 2, 128, 6144], fp16, kind="ExternalInput")
    t12d = nc.dram_tensor("t12d", [n_layers, NH // 2, 128, 2048], fp16, kind="ExternalInput")
    bqkd = nc.dram_tensor("bqkd", [n_layers, 128, 12], f32, kind="ExternalInput")
    bgd = nc.dram_tensor("bgd", [n_layers, 128, 6], f32, kind="ExternalInput")
    bvgd = nc.dram_tensor("bvgd", [n_layers, 1, H], fp16, kind="ExternalInput")
    boutd = nc.dram_tensor("boutd", [n_layers, 1, H], fp16, kind="ExternalInput")
    wsumd = nc.dram_tensor("wsumd", [n_layers, 1, 1536], fp16, kind="ExternalInput")
    x0d = nc.dram_tensor("x0d", [NQT, 128, H], f32, kind="ExternalInput")
    maskd = nc.dram_tensor("maskd", [128, NQT], f32, kind="ExternalInput")
    yd = nc.dram_tensor("yd", [NQT, 128, H], f32, kind="ExternalOutput")

    # dram scratch for positional C blocks (per layer, reused); all NQT
    # blocks of one head live contiguously per row so the write is a single
    # large-element DMA
    CROW = NQT * WEXP
    c1d = nc.dram_tensor("c1d", [2, NH, 128, CROW], fp16, kind="Internal")
    c2d = nc.dram_tensor("c2d", [2, NH, 128, CROW], fp16, kind="Internal")

    with tile.TileContext(nc) as tc:
        import contextlib
        ctx = contextlib.ExitStack()
        with ctx:
            pp = ctx.enter_context(tc.tile_pool(name="persist", bufs=1))
            sb = ctx.enter_context(tc.tile_pool(name="work", bufs=2))
            sb3 = ctx.enter_context(tc.tile_pool(name="work3", bufs=3))
            wpool = ctx.enter_context(tc.tile_pool(name="wts", bufs=4))
            ps_mm = ctx.enter_context(tc.tile_pool(name="psmm", bufs=3, space="PSUM"))
            ps_aux = ctx.enter_context(tc.tile_pool(name="psaux", bufs=1, space="PSUM"))
            ps_ctx = ctx.enter_context(tc.tile_pool(name="psctx", bufs=2, space="PSUM"))

            # persistent tiles
            x = [pp.tile([128, H], f32, name=f"x{qt}") for qt in range(NQT)]
            ident = pp.tile([128, 128], fp16, name="ident")
            make_identity(nc, ident[:])
            ones_col = pp.tile([128, 1], bf16, name="ones_col")
            nc.gpsimd.memset(ones_col[:], 1.0)
            ones_col16 = pp.tile([128, 1], fp16, name="ones_col16")
            nc.gpsimd.memset(ones_col16[:], 1.0)
            ones_row = pp.tile([1, 128], fp16, name="ones_row")
            nc.gpsimd.memset(ones_row[:], 1.0)
            ones_row32 = pp.tile([1, 64], f32, name="ones_row32")
            nc.gpsimd.memset(ones_row32[:], 1.0)
            one_f32 = pp.tile([1, 1], f32, name="one_f32")
            nc.gpsimd.memset(one_f32[:], 1.0)
            maskb = pp.tile([128, NQT], f32, name="maskb")
            nc.sync.dma_start(maskb[:], maskd[:])
            for qt in range(NQT):
                nc.sync.dma_start(x[qt][:], x0d[qt, :, :])

            # ---------------- helpers ----------------
            i32 = mybir.dt.int32

            def rsqrt_dve(dst, src, n, tagp):
                """dst[128, n] f32 = 1/sqrt(src[128, n]); DVE only (no ACT
                table). Quake seed + Newton iteration (~4e-6 rel err)."""
                nc.vector.tensor_scalar(
                    dst[:].bitcast(i32), src[:].bitcast(i32), 1, None,
                    op0=mybir.AluOpType.logical_shift_right)
                nc.vector.tensor_scalar(
                    dst[:].bitcast(i32), dst[:].bitcast(i32), -1, RSQRT_MAGIC,
                    op0=MULT, op1=ADD)
                tmp = sb.tile([128, n], f32, tag=f"{tagp}rt")
                for _ in range(1):
                    nc.vector.tensor_tensor(tmp[:], dst[:], dst[:], op=MULT)
                    nc.vector.tensor_tensor(tmp[:], tmp[:], src[:], op=MULT)
                    nc.vector.tensor_scalar(tmp[:], tmp[:], -0.5, 1.5,
                                            op0=MULT, op1=ADD)
                    nc.vector.tensor_tensor(dst[:], dst[:], tmp[:], op=MULT)

            def ln_group(tiles, width, tagp, out_dtype=fp16, inplace=False):
                """Batched LayerNorm over len(tiles) tiles [128, width].
                Stats on DVE (incl. rsqrt), apply on ACT (Identity; no table
                switch). Returns normalized fp16 tiles."""
                n = len(tiles)
                mv = sb.tile([128, 2 * n], f32, tag=f"{tagp}mv")
                for i, t in enumerate(tiles):
                    if width == H:
                        chunks = [t[:, 0:384], t[:, 384:768]]
                    else:
                        chunks = [t[:, c * 512:(c + 1) * 512]
                                  for c in range(width // 512)]
                    stats = sb.tile([128, 6 * len(chunks)], f32, tag=f"{tagp}st")
                    for j, cap in enumerate(chunks):
                        nc.vector.bn_stats(stats[:, j * 6:(j + 1) * 6], cap)
                    nc.vector.bn_aggr(mv[:, 2 * i:2 * i + 2], stats[:])
                veps = sb.tile([128, n], f32, tag=f"{tagp}ve")
                for i in range(n):
                    nc.vector.tensor_scalar_add(veps[:, i:i + 1],
                                                mv[:, 2 * i + 1:2 * i + 2], EPS)
                rstd = sb.tile([128, n], f32, tag=f"{tagp}rs")
                rsqrt_dve(rstd, veps, n, tagp)
                outs = []
                for i, t in enumerate(tiles):
                    negb = sb.tile([128, 1], f32, tag=f"{tagp}nb")
                    nc.vector.scalar_tensor_tensor(
                        negb[:], mv[:, 2 * i:2 * i + 1], -1.0, rstd[:, i:i + 1],
                        op0=MULT, op1=MULT)
                    if inplace:
                        o = t
                    else:
                        o = sb.tile([128, width], out_dtype, tag=f"{tagp}{i}",
                                    name=f"{tagp}{i}", bufs=1)
                    nc.scalar.activation(o[:], t[:], IDENT,
                                         bias=negb[:], scale=rstd[:, i:i + 1])
                    outs.append(o)
                return outs

            def transpose_h(tiles_fp16, nh_tiles, tag):
                """[128,q tiles][*, nh_tiles*128 wide] -> nh_tiles x [128, 512]."""
                outs = []
                for hc in range(nh_tiles):
                    pt = ps_mm.tile([128, 512], f32, tag="mm")
                    ptv = pt[:, 0:256].bitcast(fp16)
                    for qt in range(NQT):
                        nc.tensor.transpose(ptv[:, qt * 128:(qt + 1) * 128],
                                            tiles_fp16[qt][:, hc * 128:(hc + 1) * 128],
                                            ident[:])
                    o = sb.tile([128, 512], fp16, tag=f"hT{hc}", name=f"{tag}{hc}", bufs=1)
                    nc.scalar.copy(o[:], ptv[:])
                    outs.append(o)
                return outs

            def stats_to_cols(sum_ps, sqs_ps, nfeat, tagp):
                """[1,512] PSUM sums/sumsqs -> per-q-tile [128,4] columns of
                rstd*mean and -rstd (for folding LN into a matmul epilogue)."""
                srow = sb.tile([1, 1024], f32, tag=f"{tagp}sr")
                nc.vector.tensor_copy(srow[:, 0:512], sum_ps)
                nc.scalar.copy(srow[:, 512:1024], sqs_ps)
                stT_t = ps_aux.tile([128, 128], f32, tag="aux", bufs=2)
                stT = stT_t[:, 0:8]
                for qt in range(NQT):
                    nc.tensor.transpose(stT[:, qt:qt + 1],
                                        srow[:, qt * 128:(qt + 1) * 128],
                                        one_f32[:])
                    nc.tensor.transpose(stT[:, 4 + qt:5 + qt],
                                        srow[:, 512 + qt * 128:512 + (qt + 1) * 128],
                                        one_f32[:])
                st = sb.tile([128, 8], f32, tag=f"{tagp}stc")
                nc.vector.tensor_copy(st[:], stT)
                mean = sb.tile([128, 4], f32, tag=f"{tagp}mn")
                nc.vector.tensor_scalar(mean[:], st[:, 0:4], 1.0 / nfeat, None,
                                        op0=MULT)
                var = sb.tile([128, 4], f32, tag=f"{tagp}vr")
                nc.vector.tensor_tensor(var[:], mean[:], mean[:], op=MULT)
                nc.vector.scalar_tensor_tensor(var[:], st[:, 4:8], 1.0 / nfeat,
                                               var[:], op0=MULT, op1=SUB)
                nc.vector.tensor_scalar_add(var[:], var[:], EPS)
                rstd = sb.tile([128, 4], f32, tag=f"{tagp}rsd")
                rsqrt_dve(rstd, var, 4, tagp)
                negrstd = sb.tile([128, 4], f32, tag=f"{tagp}nr")
                nc.vector.tensor_scalar(negrstd[:], rstd[:], -1.0, None, op0=MULT)
                return mean, negrstd

            # ---------------- layers ----------------
            # passes>1 repeats the full forward (x0 reload -> 12 layers ->
            # y store) back-to-back inside one NEFF; used only by the timing
            # harness to measure marginal per-execution HW time.
            for gi in range(passes * n_layers):
                li = gi % n_layers
                if li == 0 and gi > 0:
                    for qt in range(NQT):
                        nc.sync.dma_start(yd[qt, :, :], x[qt][:])
                    for qt in range(NQT):
                        nc.sync.dma_start(x[qt][:], x0d[qt, :, :])
                par = li % 2
                # ---- attention input LN + transpose ----
                hs = ln_group(x, H, "hs")                   # 4 x [128,768] fp16
                hsT = transpose_h(hs, NHT, "hsT")           # 6 x [128,512] fp16

                # ---- QK^T + gate projections, interleaved per head-pair with
                # the positional C-block expansion so the first head's DRAM
                # round trip overlaps the remaining projections. One staging
                # copy per block, one DMA per 2 blocks; the skew reads for
                # head h are issued right after h's writes on the SAME ring.
                qkT = [None] * 12
                gT = [None] * 6
                bqk_sb = sb.tile([128, 12], f32, tag="bqk")
                nc.sync.dma_start(bqk_sb[:], bqkd[li, :, :])
                bg_sb = sb.tile([128, 6], f32, tag="bg")
                nc.sync.dma_start(bg_sb[:], bgd[li, :, :])
                wv = wpool.tile([128, 4608], fp16, tag="wvg", bufs=1)
                nc.scalar.dma_start(wv[:], wvg[li, :, :])
                bv = wpool.tile([1, H], fp16, tag="bvg", bufs=2)
                nc.sync.dma_start(bv[:], bvgd[li, :, :])
                c2p_tiles = {}
                p2c_tiles = {}
                for pr in range(NH // 2):
                    wq = wpool.tile([128, 2304], fp16, tag="wqk", bufs=2)
                    eng = nc.sync if pr % 2 == 0 else nc.scalar
                    eng.dma_start(wq[:], wqk[li, pr, :, :])
                    for kk, ot in enumerate((pr, 6 + pr, 12 + pr)):
                        po = ps_mm.tile([128, 512], f32, tag="mm")
                        for hc in range(NHT):
                            nc.tensor.matmul(po[:],
                                             wq[:, kk * 768 + hc * 128:kk * 768 + (hc + 1) * 128],
                                             hsT[hc][:],
                                             start=(hc == 0), stop=(hc == NHT - 1))
                        if ot < 12:
                            o = sb.tile([128, 512], fp16, tag=f"tp{ot}",
                                        name=f"qkT{ot}", bufs=1)
                            sc = SCALE if ot < 6 else 1.0
                            nc.scalar.activation(o[:], po[:], IDENT,
                                                 bias=bqk_sb[:, ot:ot + 1], scale=sc)
                            qkT[ot] = o
                        else:
                            # gate gelu folded into the epilogue: one ACT op,
                            # and all gate gelus complete before the softmax
                            # exps so the ACT table set switches only twice
                            # per layer.
                            o = sb.tile([128, 512], fp16, tag=f"gT{ot - 12}",
                                        name=f"gT{ot - 12}", bufs=1)
                            nc.scalar.activation(o[:], po[:], GELU,
                                                 bias=bg_sb[:, ot - 12:ot - 11])
                            gT[ot - 12] = o
                    t12_sb = sb3.tile([128, 2048], fp16, tag="t12", bufs=2)
                    teng = nc.sync if pr % 2 == 0 else nc.scalar
                    teng.dma_start(t12_sb[:], t12d[li, pr, :, :])
                    t1_sb = t12_sb[:, 0:1024]
                    t2_sb = t12_sb[:, 1024:2048]
                    for sub in range(2):
                        h = 2 * pr + sub
                        hp = sub * 64
                        qT_h = qkT[pr][hp:hp + 64, :]
                        kT_h = qkT[6 + pr][hp:hp + 64, :]
                        for tsb, lhs_full, cdram, eng in (
                                (t1_sb, qT_h, c1d, nc.sync),
                                (t2_sb, kT_h, c2d, nc.scalar)):
                            stg = sb3.tile([128, CROW], fp16,
                                           tag="cstg", bufs=2)
                            for bt in range(NQT):
                                j0 = 384 - 128 * bt
                                pa = ps_mm.tile([128, 512], f32, tag="mm")
                                nc.tensor.matmul(
                                    pa[:],
                                    lhs_full[:, bt * 128:(bt + 1) * 128],
                                    tsb[hp:hp + 64, j0:j0 + 512],
                                    start=True, stop=True)
                                pb_t = ps_aux.tile([128, 128], f32,
                                                   tag="aux", bufs=2)
                                pb = pb_t[:, 0:128]
                                nc.tensor.matmul(
                                    pb,
                                    lhs_full[:, bt * 128:(bt + 1) * 128],
                                    tsb[hp:hp + 64, j0 + 512:j0 + WEXP],
                                    start=True, stop=True)
                                # balance the PSUM->SBUF staging copies
                                # across DVE and ACT (Pool cannot read PSUM)
                                if bt % 2 == 0:
                                    nc.vector.tensor_copy(
                                        stg[:, bt * WEXP:bt * WEXP + 512], pa[:])
                                    nc.scalar.copy(
                                        stg[:, bt * WEXP + 512:(bt + 1) * WEXP], pb)
                                else:
                                    nc.scalar.copy(
                                        stg[:, bt * WEXP:bt * WEXP + 512], pa[:])
                                    nc.vector.tensor_copy(
                                        stg[:, bt * WEXP + 512:(bt + 1) * WEXP], pb)
                            eng.dma_start(cdram[par, h, :, :], stg[:])
                        # merged skew reads (all 4 q/k tiles in one DMA each),
                        # same ring as the writes they depend on
                        cbase = (par * NH + h) * 128 * CROW
                        c2p_t = sb3.tile([128, 2048], fp16, tag="c2p",
                                         name="c2p", bufs=2)
                        nc.sync.dma_start(
                            c2p_t[:],
                            bass.AP(c1d, cbase + 127,
                                    [[CROW - 1, 128], [WEXP, 4], [1, 512]]))
                        c2p_tiles[h] = c2p_t
                        p2c_t = sb3.tile([128, 2048], fp16, tag="p2c",
                                         name="p2c", bufs=2)
                        nc.scalar.dma_start(
                            p2c_t[:],
                            bass.AP(c2d, cbase + 127,
                                    [[CROW - 1, 128], [WEXP, 4], [1, 512]]))
                        p2c_tiles[h] = p2c_t

                # ---- V projection: natural layout [tok, o]; runs while the
                # first heads' C blocks make their DRAM round trip ----
                v_sb = [sb.tile([128, H], bf16, tag=f"v{tt}", name=f"v{tt}", bufs=1)
                        for tt in range(NQT)]
                for lo, w in ((0, 512), (512, 256)):
                    for tt in range(NQT):
                        po = ps_mm.tile([128, 512], f32, tag="mm")
                        for hc in range(NHT):
                            nc.tensor.matmul(po[:, :w],
                                             hsT[hc][:, tt * 128:(tt + 1) * 128],
                                             wv[:, hc * 768 + lo:hc * 768 + lo + w],
                                             start=(hc == 0), stop=False,
                                             skip_group_check=True)
                        nc.tensor.matmul(po[:, :w], ones_row[:], bv[:, lo:lo + w],
                                         start=False, stop=True,
                                         skip_group_check=True)
                        nc.scalar.copy(v_sb[tt][:, lo:lo + w], po[:, :w])

                # ---- attention, per head-pair; ctx kept transposed ----
                ctxg = []       # 6 x [128, 512] fp16: ctx^T / den
                for pr in range(NH // 2):
                    ctxP_ps = ps_ctx.tile([128, 512], f32, tag="ctxT", bufs=1)
                    recb_ps = ps_ctx.tile([128, 512], f32, tag="recb")
                    for sub in range(2):
                        h = 2 * pr + sub
                        hp = sub * 64
                        tpos = (0, hp) if hp else None
                        qT_h = qkT[pr][hp:hp + 64, :]
                        kT_h = qkT[6 + pr][hp:hp + 64, :]
                        c2p_t = c2p_tiles[h]
                        p2c_t = p2c_tiles[h]
                        # scores: c2c + positional terms accumulate in PSUM;
                        # mask goes in as the Exp bias.
                        den_t = ps_ctx.tile([128, 512], f32, tag="recb")
                        den_ps = den_t[0:1, :]
                        for kt in range(NQT):
                            ps_s = ps_mm.tile([128, 512], f32, tag="mm")
                            nc.tensor.matmul(ps_s[:],
                                             kT_h[:, kt * 128:(kt + 1) * 128],
                                             qT_h[:], start=True, stop=True)
                            for qt in range(NQT):
                                nc.tensor.matmul(
                                    ps_s[:, qt * 128:(qt + 1) * 128],
                                    c2p_t[:, qt * 512 + kt * 128:qt * 512 + (kt + 1) * 128],
                                    ident[:], start=False, stop=True,
                                    skip_group_check=True)
                            nc.tensor.matmul(ps_s[:], ident[:],
                                             p2c_t[:, kt * 512:(kt + 1) * 512],
                                             start=False, stop=True,
                                             skip_group_check=True)
                            pT = sb3.tile([128, 512], bf16, tag="pT", name="pT")
                            nc.scalar.activation(pT[:], ps_s[:], EXPF,
                                                 bias=maskb[:, kt:kt + 1])
                            nc.tensor.matmul(den_ps, ones_col[:], pT[:],
                                             start=(kt == 0), stop=(kt == NQT - 1),
                                             skip_group_check=True)
                            nc.tensor.matmul(ctxP_ps[hp:hp + 64, :],
                                             v_sb[kt][:, h * 64:(h + 1) * 64],
                                             pT[:],
                                             start=(kt == 0), stop=(kt == NQT - 1),
                                             skip_group_check=True,
                                             tile_position=tpos)
                        rec = sb.tile([1, 512], f32, tag="rec")
                        nc.vector.reciprocal_approx_fast(rec[:], den_ps)
                        nc.tensor.matmul(recb_ps[hp:hp + 64, :], ones_row32[:],
                                         rec[:], start=True, stop=True,
                                         skip_group_check=True,
                                         tile_position=tpos)
                    # cg = ctx^T * gelu(g) * (1/den), all in [feature, token]
                    # (two TTs: each may read only one PSUM operand)
                    cx = sb.tile([128, 512], fp16, tag="cx", bufs=2)
                    nc.vector.tensor_tensor(cx[:], ctxP_ps[:], gT[pr][:],
                                            op=MULT)
                    cg = sb.tile([128, 512], fp16, tag=f"cg{pr}",
                                 name=f"cg{pr}", bufs=1)
                    nc.vector.tensor_tensor(cg[:], cx[:], recb_ps[:], op=MULT)
                    ctxg.append(cg)

                # ---- cg LN stats (partition-axis, via ones-matmuls) ----
                cgT = ctxg      # 6 x [128, 512] fp16: ctx^T * gelu(g) / den
                sum_t = ps_ctx.tile([128, 512], f32, tag="recb")
                sqs_t = ps_ctx.tile([128, 512], f32, tag="recb")
                sum_ps, sqs_ps = sum_t[0:1, :], sqs_t[0:1, :]
                for pr in range(NH // 2):
                    cg = cgT[pr]
                    sq = sb.tile([128, 512], fp16, tag=f"gT{pr}",
                                 name=f"sq{pr}", bufs=1)
                    nc.gpsimd.tensor_tensor(sq[:], cg[:], cg[:], op=MULT)
                    nc.tensor.matmul(sum_ps, ones_col16[:], cg[:],
                                     start=(pr == 0), stop=(pr == NHT - 1),
                                     skip_group_check=True)
                    nc.tensor.matmul(sqs_ps, ones_col16[:], sq[:],
                                     start=(pr == 0), stop=(pr == NHT - 1),
                                     skip_group_check=True)
                rmean_c, negrstd_c = stats_to_cols(sum_ps, sqs_ps, H, "cgs")

                # broadcast rows: ones (x) Woutsum, ones (x) bout
                wsum_sb = wpool.tile([1, 1536], fp16, tag="wsum", bufs=2)
                nc.sync.dma_start(wsum_sb[:], wsumd[li, :, :])
                bo = wpool.tile([1, H], fp16, tag="bout", bufs=2)
                nc.sync.dma_start(bo[:], boutd[li, :, :])
                wob_sb = sb.tile([128, H], fp16, tag="wob", bufs=1)
                bb_sb = sb.tile([128, H], fp16, tag="bb", bufs=1)
                for lo, w in ((0, 512), (512, 256)):
                    pw = ps_mm.tile([128, 512], f32, tag="mm")
                    nc.tensor.matmul(pw[:, :w], ones_row[:], wsum_sb[:, lo:lo + w],
                                     start=True, stop=True, skip_group_check=True)
                    nc.scalar.copy(wob_sb[:, lo:lo + w], pw[:, :w])
                    pb2 = ps_mm.tile([128, 512], f32, tag="mm")
                    nc.tensor.matmul(pb2[:, :w], ones_row[:], bo[:, lo:lo + w],
                                     start=True, stop=True, skip_group_check=True)
                    nc.scalar.copy(bb_sb[:, lo:lo + w], pb2[:, :w])

                # ---- out proj from cgT with LN folded into the epilogue ----
                wo = wpool.tile([128, 4608], fp16, tag="wout", bufs=1)
                nc.scalar.dma_start(wo[:], wout[li, :, :])
                for qt in range(NQT):
                    nc.vector.tensor_add(x[qt][:], x[qt][:], bb_sb[:])
                    veng = nc.vector
                    for lo, w in ((0, 512), (512, 256)):
                        po = ps_mm.tile([128, 512], f32, tag="mm")
                        for hc in range(NHT):
                            nc.tensor.matmul(po[:, :w],
                                             cgT[hc][:, qt * 128:(qt + 1) * 128],
                                             wo[:, hc * 768 + lo:hc * 768 + lo + w],
                                             start=(hc == 0), stop=(hc == NHT - 1),
                                             skip_group_check=True)
                        pe = sb.tile([128, 512], fp16, tag="poev")
                        nc.scalar.copy(pe[:, :w], po[:, :w])
                        t = sb.tile([128, 512], f32, tag="fold")
                        veng.scalar_tensor_tensor(
                            t[:, :w], wob_sb[:, lo:lo + w], rmean_c[:, qt:qt + 1],
                            pe[:, :w], op0=MULT, op1=SUB)
                        veng.scalar_tensor_tensor(
                            x[qt][:, lo:lo + w], t[:, :w], negrstd_c[:, qt:qt + 1],
                            x[qt][:, lo:lo + w], op0=MULT, op1=ADD)

                # ---- FFN (transposed: W1 emits [feature, token]) ----
                h2 = ln_group(x, H, "hs")
                h2T = transpose_h(h2, NHT, "h2T")
                a_tiles = [sb.tile([128, 512], fp16, tag=f"tp{i}", name=f"a{i}",
                                   bufs=1) for i in range(16)]
                sum2_t = ps_ctx.tile([128, 512], f32, tag="recb")
                sqs2_t = ps_ctx.tile([128, 512], f32, tag="recb")
                sum2_ps, sqs2_ps = sum2_t[0:1, :], sqs2_t[0:1, :]
                for g8 in range(8):
                    # half-blob loads: finer-grained prefetch pipeline
                    wf_h = []
                    for wh in range(2):
                        t = wpool.tile([128, 1536], fp16, tag="w1", bufs=3)
                        eng = nc.sync if (2 * g8 + wh) % 2 == 0 else nc.scalar
                        eng.dma_start(
                            t[:], w1[li, g8, :, wh * 1536:(wh + 1) * 1536])
                        wf_h.append(t)
                    for otl in range(4):
                        ot = g8 * 4 + otl
                        wf = wf_h[otl // 2]
                        wlo = (otl % 2) * 768
                        po = ps_mm.tile([128, 512], f32, tag="mm")
                        for hc in range(NHT):
                            nc.tensor.matmul(
                                po[:],
                                wf[:, wlo + hc * 128:wlo + (hc + 1) * 128],
                                h2T[hc][:],
                                start=(hc == 0), stop=(hc == NHT - 1))
                        if ot < 16:
                            nc.scalar.copy(a_tiles[ot][:], po[:])
                        else:
                            # native gelu straight out of PSUM (table already
                            # on the gelu set since the gate gelus above)
                            at = a_tiles[ot - 16]
                            gt = sb.tile([128, 512], fp16, tag="ffng")
                            nc.scalar.activation(gt[:], po[:], GELU)
                            nc.vector.tensor_mul(at[:], at[:], gt[:])
                            # tile (ot-16) is final: accumulate its LN stats now
                            i = ot - 16
                            nc.tensor.matmul(sum2_ps, ones_col16[:], at[:],
                                             start=(i == 0), stop=(i == 15),
                                             skip_group_check=True)
                            sq = sb.tile([128, 512], fp16, tag="sq")
                            nc.gpsimd.tensor_tensor(sq[:], at[:], at[:], op=MULT)
                            nc.tensor.matmul(sqs2_ps, ones_col16[:], sq[:],
                                             start=(i == 0), stop=(i == 15),
                                             skip_group_check=True)

                # ---- W2 halves (loaded early so the DMA overlaps stats) ----
                w2h = []
                for hh in range(2):
                    wt2 = wpool.tile([128, 6144], fp16, tag=f"w2h{hh}", bufs=1)
                    eng = nc.sync if hh == 0 else nc.scalar
                    eng.dma_start(wt2[:], w2[li, hh, :, :])
                    w2h.append(wt2)

                rmean2, negrstd2 = stats_to_cols(sum2_ps, sqs2_ps, FI, "uns")
                w2b_sb = sb.tile([128, H], fp16, tag="w2b", bufs=1)
                for lo, w in ((0, 512), (512, 256)):
                    pw = ps_mm.tile([128, 512], f32, tag="mm")
                    nc.tensor.matmul(pw[:, :w], ones_row[:],
                                     wsum_sb[:, 768 + lo:768 + lo + w],
                                     start=True, stop=True, skip_group_check=True)
                    nc.scalar.copy(w2b_sb[:, lo:lo + w], pw[:, :w])

                # ---- W2 from raw GeGLU tiles with LN folded in ----
                for qt in range(NQT):
                    veng = nc.vector
                    for lo, w in ((0, 512), (512, 256)):
                        po = ps_mm.tile([128, 512], f32, tag="mm")
                        for ic in range(16):
                            nc.tensor.matmul(
                                po[:, :w], a_tiles[ic][:, qt * 128:(qt + 1) * 128],
                                w2h[ic // 8][:, (ic % 8) * 768 + lo:(ic % 8) * 768 + lo + w],
                                start=(ic == 0), stop=(ic == 15),
                                skip_group_check=True)
                        pe = sb.tile([128, 512], fp16, tag="poev")
                        nc.scalar.copy(pe[:, :w], po[:, :w])
                        t = sb.tile([128, 512], f32, tag="fold")
                        veng.scalar_tensor_tensor(
                            t[:, :w], w2b_sb[:, lo:lo + w], rmean2[:, qt:qt + 1],
                            pe[:, :w], op0=MULT, op1=SUB)
                        veng.scalar_tensor_tensor(
                            x[qt][:, lo:lo + w], t[:, :w], negrstd2[:, qt:qt + 1],
                            x[qt][:, lo:lo + w], op0=MULT, op1=ADD)

            # ---- output ----
            for qt in range(NQT):
                nc.sync.dma_start(yd[qt, :, :], x[qt][:])

    nc.finalize()
    return nc


_CACHE = {}


def _get_nc(n_layers, passes=1):
    key = (n_layers, passes)
    if key not in _CACHE:
        _CACHE[key] = _build(n_layers, passes)
    return _CACHE[key]


# ---------------------------------------------------------------- host prep
def _prep_shared(word_emb, rel_emb, rel_g, rel_b, Wqk, bqk, Wvg, bvg, Wout,
                 bout, W1, W2, n_layers):
    beta = _beta_delta()                     # [1023]
    idx_c2p = beta[1022 - np.arange(1023)]   # T1: delta = 511 - j
    idx_p2c = beta[np.arange(1023)]          # T2: delta = j - 511
    rel = _ln_np(rel_emb.astype(np.float64)).astype(np.float32) * rel_g + rel_b

    d = {}
    t1 = np.zeros((n_layers, NH, 64, 1024), np.float32)  # packed to pairs below
    t2 = np.zeros((n_layers, NH, 64, 1024), np.float32)
    wqk_t = np.zeros((n_layers, 18, 128, 768), np.float32)  # regrouped below
    wvg_t = np.zeros((n_layers, 128, 4608), np.float32)
    wout_t = np.zeros((n_layers, 128, 4608), np.float32)
    w1_t = np.zeros((n_layers, 8, 128, 3072), np.float32)
    w2_t = np.zeros((n_layers, 2, 128, 6144), np.float32)
    bqk_t = np.zeros((n_layers, 128, 12), np.float32)
    bg_t = np.zeros((n_layers, 128, 6), np.float32)
    bvg_t = np.zeros((n_layers, 1, H), np.float32)
    bout_t = np.zeros((n_layers, 1, H), np.float32)
    wsum_t = np.zeros((n_layers, 1, 1536), np.float32)
    for li in range(n_layers):
        pos = rel @ Wqk[li].T + bqk[li]          # [63, 1536]
        qpos = pos[:, :H].reshape(63, NH, 64)
        kpos = pos[:, H:].reshape(63, NH, 64)
        # T1[j] = kpos[beta(511-j)], T2[j] = qpos[beta(j-511)] * SCALE
        t1[li, :, :, :1023] = kpos[idx_c2p].transpose(1, 2, 0)
        t2[li, :, :, :1023] = qpos[idx_p2c].transpose(1, 2, 0) * SCALE

        wqkT = Wqk[li].T.copy()                  # [768, 1536]
        # qk blobs: [ot, p, hc*128+oo] = wqkT[hc*128+p, ot*128+oo]
        wqk_t[li, :12] = (wqkT.reshape(NHT, 128, 12, 128)
                          .transpose(2, 1, 0, 3).reshape(12, 128, 768))
        wvgT = Wvg[li].T.copy()                  # [768, 1536]
        # gate blobs (transposed proj): [got, p, hc*128+oo]
        wqk_t[li, 12:] = (wvgT[:, H:].reshape(NHT, 128, NHT, 128)
                          .transpose(2, 1, 0, 3).reshape(6, 128, 768))
        # V rhs blob: [p, hc*768+c] = wvgT[hc*128+p, c]
        wvg_t[li] = (wvgT[:, :H].reshape(NHT, 128, H)
                     .transpose(1, 0, 2).reshape(128, 4608))
        wout_t[li] = (Wout[li].T.reshape(NHT, 128, H)
                      .transpose(1, 0, 2).reshape(128, 4608))
        # W1 lhsT blobs (transposed proj): [g, p, otl*768 + hc*128 + oo]
        w1_t[li] = (W1[li].T.reshape(NHT, 128, 32, 128)
                    .transpose(2, 1, 0, 3).reshape(8, 4, 128, NHT * 128)
                    .transpose(0, 2, 1, 3).reshape(8, 128, 3072))
        w2_t[li] = (W2[li].T.reshape(2, 8, 128, H)
                    .transpose(0, 2, 1, 3).reshape(2, 128, 6144))
        bqk_t[li] = bqk[li].reshape(12, 128).T
        bg_t[li] = bvg[li][H:].reshape(6, 128).T
        bvg_t[li, 0] = bvg[li][:H]
        bout_t[li, 0] = bout[li]
        wsum_t[li, 0, :H] = Wout[li].sum(axis=1)
        wsum_t[li, 0, H:] = W2[li].sum(axis=1)

    # regroup: [li, pr, :, 0:768|768:1536|1536:2304] = ots (pr, 6+pr, 12+pr)
    wqk_g = np.concatenate([wqk_t[:, 0:6], wqk_t[:, 6:12], wqk_t[:, 12:18]],
                           axis=3)
    d["wqk"] = wqk_g.astype(F16)
    d["wvg"] = wvg_t.astype(F16)
    d["wout"] = wout_t.astype(F16)
    d["w1"] = w1_t.astype(F16)
    d["w2"] = w2_t.astype(F16)
    d["t12d"] = np.concatenate(
        [t1.reshape(n_layers, NH // 2, 128, 1024),
         t2.reshape(n_layers, NH // 2, 128, 1024)], axis=3).astype(F16)
    d["bqkd"] = bqk_t
    d["bgd"] = bg_t
    d["bvgd"] = bvg_t.astype(F16)
    d["boutd"] = bout_t.astype(F16)
    d["wsumd"] = wsum_t.astype(F16)
    return d


def _make_in_maps(inputs, n_layers):
    input_ids = np.asarray(inputs["input_ids"])
    attention_mask = np.asarray(inputs["attention_mask"])
    word_emb = np.asarray(inputs["word_emb"], np.float32)

    shared = _prep_shared(
        word_emb, np.asarray(inputs["rel_emb"], np.float32),
        np.asarray(inputs["rel_g"], np.float32), np.asarray(inputs["rel_b"], np.float32),
        np.asarray(inputs["Wqk"], np.float32), np.asarray(inputs["bqk"], np.float32),
        np.asarray(inputs["Wvg"], np.float32), np.asarray(inputs["bvg"], np.float32),
        np.asarray(inputs["Wout"], np.float32), np.asarray(inputs["bout"], np.float32),
        np.asarray(inputs["W1"], np.float32), np.asarray(inputs["W2"], np.float32),
        n_layers)

    in_maps = []
    for b in range(B):
        m = dict(shared)
        x0 = _ln_np(word_emb[input_ids[:, b]].astype(np.float64)).astype(np.float32)
        m["x0d"] = x0.reshape(NQT, 128, H)
        mb = np.where(attention_mask[b, 0, 0, :], MASK_NEG, 0.0).astype(np.float32)
        m["maskd"] = mb.reshape(NQT, 128).T.copy()
        in_maps.append(m)
    return in_maps


def run(inputs, n_layers=L, trace=False):
    nc = _get_nc(n_layers)
    in_maps = _make_in_maps(inputs, n_layers)
    res = run_bass_kernel_spmd(nc, in_maps, core_ids=list(range(B)), trace=trace)
    out = np.zeros((S, B, H), np.float32)
    for b in range(B):
        out[:, b, :] = res.results[b]["yd"].reshape(S, H)
    return out, res


def kernel(**inputs) -> np.ndarray:
    out, _ = run(inputs, L)
    return out


# ------------------------------------------------------- timing-only runner
def make_timed_runner(n_layers, in_maps, passes=1):
    """Build a persistent jitted PJRT callable over 8 cores for wall-clock
    timing (the axon NTFF profile hook is unavailable in this container).
    passes>1 builds the NEFF that repeats the full forward back-to-back;
    timing two pass counts and taking the slope cancels the fixed axon/PJRT
    dispatch overhead, leaving the true per-execution HW time."""
    import jax
    from jax.sharding import Mesh, PartitionSpec, NamedSharding
    from jax.experimental.shard_map import shard_map
    from concourse import bass2jax

    nc = _get_nc(n_layers, passes)
    bass2jax.install_neuronx_cc_hook()
    partition_name = nc.partition_id_tensor.name if nc.partition_id_tensor else None
    in_names, out_names, out_avals, zero_outs = [], [], [], []
    import concourse.mybir as _mb
    for alloc in nc.m.functions[0].allocations:
        if not isinstance(alloc, _mb.MemoryLocationSet):
            continue
        name = alloc.memorylocations[0].name
        if alloc.kind == "ExternalInput":
            if name != partition_name:
                in_names.append(name)
        elif alloc.kind == "ExternalOutput":
            out_names.append(name)
            shape = tuple(alloc.tensor_shape)
            dtype = _mb.dt.np(alloc.dtype)
            out_avals.append(jax.core.ShapedArray(shape, dtype))
            zero_outs.append(np.zeros(shape, dtype))
    n_params = len(in_names)
    n_outs = len(out_avals)
    all_in_names = list(in_names) + out_names
    if partition_name is not None:
        all_in_names = all_in_names + [partition_name]

    def _body(*args):
        operands = list(args)
        if partition_name is not None:
            operands.append(bass2jax.partition_id_tensor())
        outs = bass2jax._bass_exec_p.bind(
            *operands, out_avals=tuple(out_avals), in_names=tuple(all_in_names),
            out_names=tuple(out_names), lowering_input_output_aliases=(),
            sim_require_finite=True, sim_require_nnan=True, nc=nc)
        return tuple(outs)

    n_cores = B
    devices = jax.devices()[:n_cores]
    mesh = Mesh(np.asarray(devices), ("core",))
    P = PartitionSpec
    sharded = jax.jit(
        shard_map(_body, mesh=mesh, in_specs=(P("core"),) * (n_params + n_outs),
                  out_specs=(P("core"),) * n_outs, check_rep=False),
        keep_unused=True)

    concat_in = [
        np.concatenate([np.asarray(in_maps[c][nm]) for c in range(n_cores)], axis=0)
        for nm in in_names]
    concat_zeros = [np.zeros((n_cores * z.shape[0], *z.shape[1:]), z.dtype)
                    for z in zero_outs]
    shard = NamedSharding(mesh, P("core"))
    dev_in = [jax.device_put(a, shard) for a in concat_in]
    dev_zeros = [jax.device_put(a, shard) for a in concat_zeros]

    def call():
        outs = sharded(*dev_in, *dev_zeros)
        jax.block_until_ready(outs)
        return outs

    return call


# revision 7
# speedup vs baseline: 20.7751x; 1.0049x over previous
"""DeBERTa-style 12-layer transformer on 8 TRN2 NeuronCores.

Sharding: data-parallel over batch (B=8 -> 1 sequence per core, no
collectives). Weights are host-prepped (transposed/blobbed/fp16) and
replicated per core. Relative-position tables are expanded on host into
per-layer T1/T2 tables; the (q,k)-dependent gather is done on device via
matmul + a strided "skew" DMA read from a DRAM scratch buffer, and the
positional terms are accumulated straight into the score PSUM by TensorE
(c2p via transpose-accumulate, p2c via identity-lhsT accumulate).

v2: all DMAs ride the two HWDGE rings (SP + ACT) so the Pool engine is
free. The C-block expansion stores all four blocks of a head contiguously
per DRAM row, so each head/direction is one large-element write DMA and
one 4-in-1 merged skew read, issued right after the writes on the same
ring so the round trip pipelines with the previous head's scores; the
PSUM->SBUF staging copies alternate between DVE and ACT. Projections are
interleaved per head-pair with the expansion so PE stays fed. Every gelu
is a native ACT Gelu_apprx_tanh (the gate gelu is folded into the
projection epilogue), batched so the ACT table set switches only ~3x per
layer. The attention tail stays transposed ([feature, token]):
per-head-pair ctx^T accumulates in one PSUM bank, the softmax reciprocal
broadcasts via cheap fp16 ones-matmuls, and the LayerNorms of ctx*g /
GeGLU fold into the out-projection / W2 epilogues via ones-matmul
statistics. _build(n_layers, passes=K) can repeat the full forward K
times back-to-back inside one NEFF; the timing harness uses two pass
counts and takes the slope to cancel fixed dispatch overhead.
"""

import math
import numpy as np
import ml_dtypes

import concourse.bacc as bacc
import concourse.bass as bass
import concourse.mybir as mybir
from concourse import tile
from concourse.bass_utils import run_bass_kernel_spmd
from concourse.masks import make_identity

BF = ml_dtypes.bfloat16
F16 = np.float16
bf16 = mybir.dt.bfloat16
fp16 = mybir.dt.float16
f32 = mybir.dt.float32

V = 32768; H = 768; NH = 12; D = 64; L = 12; FI = 2048
S = 512; B = 8; BK = 32; MAXP = 512; EPS = 1e-7
SCALE = 1.0 / math.sqrt(3 * D)
NQT = S // 128      # 4 token tiles
NHT = H // 128      # 6 hidden tiles
WEXP = 640          # C-block width (per-tile expansion window)
MASK_NEG = -60000.0
RSQRT_MAGIC = 0x5F3759DF
MULT = mybir.AluOpType.mult
ADD = mybir.AluOpType.add
SUB = mybir.AluOpType.subtract
GELU = mybir.ActivationFunctionType.Gelu_apprx_tanh
EXPF = mybir.ActivationFunctionType.Exp
IDENT = mybir.ActivationFunctionType.Identity


# ---------------------------------------------------------------- host math
def _beta_delta():
    """bucket(delta)+31 for delta in [-511, 511], indexed by delta+511."""
    delta = np.arange(-(S - 1), S)
    sign = np.sign(delta)
    mid = BK // 2
    abs_pos = np.where((delta < mid) & (delta > -mid), mid - 1,
                       np.minimum(np.abs(delta), MAXP - 1))
    log_pos = np.ceil(np.log(abs_pos / mid) / math.log((MAXP - 1) / mid)
                      * (mid - 1)).astype(np.int64) + mid
    bucket = np.where(abs_pos <= mid, delta, log_pos * sign).astype(np.int64)
    return bucket + BK - 1


def _ln_np(x):
    m = x.mean(-1, keepdims=True)
    v = x.var(-1, keepdims=True)
    return (x - m) / np.sqrt(v + EPS)


# ---------------------------------------------------------------- builder
def _build(n_layers, passes=1):
    nc = bacc.Bacc("TRN2", target_bir_lowering=False, num_devices=B)

    # ---- dram inputs (host-prepped layouts; partition-major weight blobs) ----
    wqk = nc.dram_tensor("wqk", [n_layers, 6, 128, 2304], fp16, kind="ExternalInput")
    wvg = nc.dram_tensor("wvg", [n_layers, 128, 4608], fp16, kind="ExternalInput")
    wout = nc.dram_tensor("wout", [n_layers, 128, 4608], fp16, kind="ExternalInput")
    w1 = nc.dram_tensor("w1", [n_layers, 8, 128, 3072], fp16, kind="ExternalInput")
    w2 = nc.dram_tensor("w2", [n_layers, 2, 128, 6144], fp16, kind="ExternalInput")
    t12d = nc.dram_tensor("t12d", [n_layers, NH // 2, 128, 2048], fp16, kind="ExternalInput")
    bqkd = nc.dram_tensor("bqkd", [n_layers, 128, 12], f32, kind="ExternalInput")
    bgd = nc.dram_tensor("bgd", [n_layers, 128, 6], f32, kind="ExternalInput")
    bvgd = nc.dram_tensor("bvgd", [n_layers, 1, H], fp16, kind="ExternalInput")
    boutd = nc.dram_tensor("boutd", [n_layers, 1, H], fp16, kind="ExternalInput")
    wsumd = nc.dram_tensor("wsumd", [n_layers, 1, 1536], fp16, kind="ExternalInput")
    x0d = nc.dram_tensor("x0d", [NQT, 128, H], f32, kind="ExternalInput")
    maskd = nc.dram_tensor("maskd", [128, NQT], f32, kind="ExternalInput")
    yd = nc.dram_tensor("yd", [NQT, 128, H], f32, kind="ExternalOutput")

    # dram scratch for positional C blocks (per layer, reused); all NQT
    # blocks of one head live contiguously per row so the write is a single
    # large-element DMA
    CROW = NQT * WEXP
    c1d = nc.dram_tensor("c1d", [2, NH, 128, CROW], fp16, kind="Internal")
    c2d = nc.dram_tensor("c2d", [2, NH, 128, CROW], fp16, kind="Internal")

    with tile.TileContext(nc) as tc:
        import contextlib
        ctx = contextlib.ExitStack()
        with ctx:
            pp = ctx.enter_context(tc.tile_pool(name="persist", bufs=1))
            sb = ctx.enter_context(tc.tile_pool(name="work", bufs=2))
            sb3 = ctx.enter_context(tc.tile_pool(name="work3", bufs=3))
            wpool = ctx.enter_context(tc.tile_pool(name="wts", bufs=4))
            ps_mm = ctx.enter_context(tc.tile_pool(name="psmm", bufs=3, space="PSUM"))
            ps_aux = ctx.enter_context(tc.tile_pool(name="psaux", bufs=1, space="PSUM"))
            ps_ctx = ctx.enter_context(tc.tile_pool(name="psctx", bufs=2, space="PSUM"))

            # persistent tiles
            x = [pp.tile([128, H], f32, name=f"x{qt}") for qt in range(NQT)]
            ident = pp.tile([128, 128], fp16, name="ident")
            make_identity(nc, ident[:])
            ones_col = pp.tile([128, 1], bf16, name="ones_col")
            nc.gpsimd.memset(ones_col[:], 1.0)
            ones_col16 = pp.tile([128, 1], fp16, name="ones_col16")
            nc.gpsimd.memset(ones_col16[:], 1.0)
            ones_row = pp.tile([1, 128], fp16, name="ones_row")
            nc.gpsimd.memset(ones_row[:], 1.0)

            one_f32 = pp.tile([1, 1], f32, name="one_f32")
            nc.gpsimd.memset(one_f32[:], 1.0)
            maskb = pp.tile([128, NQT], f32, name="maskb")
            nc.sync.dma_start(maskb[:], maskd[:])
            for qt in range(NQT):
                nc.sync.dma_start(x[qt][:], x0d[qt, :, :])

            # ---------------- helpers ----------------
            i32 = mybir.dt.int32

            def rsqrt_dve(dst, src, n, tagp):
                """dst[128, n] f32 = 1/sqrt(src[128, n]); DVE only (no ACT
                table). Quake seed + Newton iteration (~4e-6 rel err)."""
                nc.vector.tensor_scalar(
                    dst[:].bitcast(i32), src[:].bitcast(i32), 1, None,
                    op0=mybir.AluOpType.logical_shift_right)
                nc.vector.tensor_scalar(
                    dst[:].bitcast(i32), dst[:].bitcast(i32), -1, RSQRT_MAGIC,
                    op0=MULT, op1=ADD)
                tmp = sb.tile([128, n], f32, tag=f"{tagp}rt")
                for _ in range(1):
                    nc.vector.tensor_tensor(tmp[:], dst[:], dst[:], op=MULT)
                    nc.vector.tensor_tensor(tmp[:], tmp[:], src[:], op=MULT)
                    nc.vector.tensor_scalar(tmp[:], tmp[:], -0.5, 1.5,
                                            op0=MULT, op1=ADD)
                    nc.vector.tensor_tensor(dst[:], dst[:], tmp[:], op=MULT)

            def ln_group(tiles, width, tagp, out_dtype=fp16, inplace=False):
                """Batched LayerNorm over len(tiles) tiles [128, width].
                Stats on DVE (incl. rsqrt), apply on ACT (Identity; no table
                switch). Returns normalized fp16 tiles."""
                n = len(tiles)
                mv = sb.tile([128, 2 * n], f32, tag=f"{tagp}mv")
                for i, t in enumerate(tiles):
                    if width == H:
                        chunks = [t[:, 0:384], t[:, 384:768]]
                    else:
                        chunks = [t[:, c * 512:(c + 1) * 512]
                                  for c in range(width // 512)]
                    stats = sb.tile([128, 6 * len(chunks)], f32, tag=f"{tagp}st")
                    for j, cap in enumerate(chunks):
                        nc.vector.bn_stats(stats[:, j * 6:(j + 1) * 6], cap)
                    nc.vector.bn_aggr(mv[:, 2 * i:2 * i + 2], stats[:])
                veps = sb.tile([128, n], f32, tag=f"{tagp}ve")
                for i in range(n):
                    nc.vector.tensor_scalar_add(veps[:, i:i + 1],
                                                mv[:, 2 * i + 1:2 * i + 2], EPS)
                rstd = sb.tile([128, n], f32, tag=f"{tagp}rs")
                rsqrt_dve(rstd, veps, n, tagp)
                outs = []
                for i, t in enumerate(tiles):
                    negb = sb.tile([128, 1], f32, tag=f"{tagp}nb")
                    nc.vector.scalar_tensor_tensor(
                        negb[:], mv[:, 2 * i:2 * i + 1], -1.0, rstd[:, i:i + 1],
                        op0=MULT, op1=MULT)
                    if inplace:
                        o = t
                    else:
                        o = sb.tile([128, width], out_dtype, tag=f"{tagp}{i}",
                                    name=f"{tagp}{i}", bufs=1)
                    nc.scalar.activation(o[:], t[:], IDENT,
                                         bias=negb[:], scale=rstd[:, i:i + 1])
                    outs.append(o)
                return outs

            def transpose_h(tiles_fp16, nh_tiles, tag):
                """[128,q tiles][*, nh_tiles*128 wide] -> nh_tiles x [128, 512]."""
                outs = []
                for hc in range(nh_tiles):
                    pt = ps_mm.tile([128, 512], f32, tag="mm")
                    ptv = pt[:, 0:256].bitcast(fp16)
                    for qt in range(NQT):
                        nc.tensor.transpose(ptv[:, qt * 128:(qt + 1) * 128],
                                            tiles_fp16[qt][:, hc * 128:(hc + 1) * 128],
                                            ident[:])
                    o = sb.tile([128, 512], fp16, tag=f"hT{hc}", name=f"{tag}{hc}", bufs=1)
                    nc.scalar.copy(o[:], ptv[:])
                    outs.append(o)
                return outs

            def stats_to_cols(sum_ps, sqs_ps, nfeat, tagp):
                """[1,512] PSUM sums/sumsqs -> per-q-tile [128,4] columns of
                rstd*mean and -rstd (for folding LN into a matmul epilogue)."""
                srow = sb.tile([1, 1024], f32, tag=f"{tagp}sr")
                nc.vector.tensor_copy(srow[:, 0:512], sum_ps)
                nc.scalar.copy(srow[:, 512:1024], sqs_ps)
                stT_t = ps_aux.tile([128, 128], f32, tag="aux", bufs=2)
                stT = stT_t[:, 0:8]
                for qt in range(NQT):
                    nc.tensor.transpose(stT[:, qt:qt + 1],
                                        srow[:, qt * 128:(qt + 1) * 128],
                                        one_f32[:])
                    nc.tensor.transpose(stT[:, 4 + qt:5 + qt],
                                        srow[:, 512 + qt * 128:512 + (qt + 1) * 128],
                                        one_f32[:])
                st = sb.tile([128, 8], f32, tag=f"{tagp}stc")
                nc.vector.tensor_copy(st[:], stT)
                mean = sb.tile([128, 4], f32, tag=f"{tagp}mn")
                nc.vector.tensor_scalar(mean[:], st[:, 0:4], 1.0 / nfeat, None,
                                        op0=MULT)
                var = sb.tile([128, 4], f32, tag=f"{tagp}vr")
                nc.vector.tensor_tensor(var[:], mean[:], mean[:], op=MULT)
                nc.vector.scalar_tensor_tensor(var[:], st[:, 4:8], 1.0 / nfeat,
                                               var[:], op0=MULT, op1=SUB)
                nc.vector.tensor_scalar_add(var[:], var[:], EPS)
                rstd = sb.tile([128, 4], f32, tag=f"{tagp}rsd")
                rsqrt_dve(rstd, var, 4, tagp)
                negrstd = sb.tile([128, 4], f32, tag=f"{tagp}nr")
                nc.vector.tensor_scalar(negrstd[:], rstd[:], -1.0, None, op0=MULT)
                return mean, negrstd

            # ---------------- layers ----------------
            # passes>1 repeats the full forward (x0 reload -> 12 layers ->
            # y store) back-to-back inside one NEFF; used only by the timing
            # harness to measure marginal per-execution HW time.
            for gi in range(passes * n_layers):
                li = gi % n_layers
                if li == 0 and gi > 0:
                    for qt in range(NQT):
                        nc.sync.dma_start(yd[qt, :, :], x[qt][:])
                    for qt in range(NQT):
                        nc.sync.dma_start(x[qt][:], x0d[qt, :, :])
                par = li % 2
                # ---- attention input LN + transpose ----
                hs = ln_group(x, H, "hs")                   # 4 x [128,768] fp16
                hsT = transpose_h(hs, NHT, "hsT")           # 6 x [128,512] fp16

                # ---- QK^T + gate projections, interleaved per head-pair with
                # the positional C-block expansion so the first head's DRAM
                # round trip overlaps the remaining projections. One staging
                # copy per block, one DMA per 2 blocks; the skew reads for
                # head h are issued right after h's writes on the SAME ring.
                qkT = [None] * 12
                gT = [None] * 6
                bqk_sb = sb.tile([128, 12], f32, tag="bqk")
                nc.sync.dma_start(bqk_sb[:], bqkd[li, :, :])
                bg_sb = sb.tile([128, 6], f32, tag="bg")
                nc.sync.dma_start(bg_sb[:], bgd[li, :, :])
                wv = wpool.tile([128, 4608], fp16, tag="wvg", bufs=1)
                nc.scalar.dma_start(wv[:], wvg[li, :, :])
                bv = wpool.tile([1, H], fp16, tag="bvg", bufs=2)
                nc.sync.dma_start(bv[:], bvgd[li, :, :])
                c2p_tiles = {}
                p2c_tiles = {}
                for pr in range(NH // 2):
                    wq = wpool.tile([128, 2304], fp16, tag="wqk", bufs=2)
                    eng = nc.sync if pr % 2 == 0 else nc.scalar
                    eng.dma_start(wq[:], wqk[li, pr, :, :])
                    for kk, ot in enumerate((pr, 6 + pr, 12 + pr)):
                        po = ps_mm.tile([128, 512], f32, tag="mm")
                        for hc in range(NHT):
                            nc.tensor.matmul(po[:],
                                             wq[:, kk * 768 + hc * 128:kk * 768 + (hc + 1) * 128],
                                             hsT[hc][:],
                                             start=(hc == 0), stop=(hc == NHT - 1))
                        if ot < 12:
                            o = sb.tile([128, 512], fp16, tag=f"tp{ot}",
                                        name=f"qkT{ot}", bufs=1)
                            sc = SCALE if ot < 6 else 1.0
                            nc.scalar.activation(o[:], po[:], IDENT,
                                                 bias=bqk_sb[:, ot:ot + 1], scale=sc)
                            qkT[ot] = o
                        else:
                            # gate gelu folded into the epilogue: one ACT op,
                            # and all gate gelus complete before the softmax
                            # exps so the ACT table set switches only twice
                            # per layer.
                            o = sb.tile([128, 512], fp16, tag=f"gT{ot - 12}",
                                        name=f"gT{ot - 12}", bufs=1)
                            nc.scalar.activation(o[:], po[:], GELU,
                                                 bias=bg_sb[:, ot - 12:ot - 11])
                            gT[ot - 12] = o
                    t12_sb = sb3.tile([128, 2048], fp16, tag="t12", bufs=2)
                    teng = nc.sync if pr % 2 == 0 else nc.scalar
                    teng.dma_start(t12_sb[:], t12d[li, pr, :, :])
                    t1_sb = t12_sb[:, 0:1024]
                    t2_sb = t12_sb[:, 1024:2048]
                    for sub in range(2):
                        h = 2 * pr + sub
                        hp = sub * 64
                        qT_h = qkT[pr][hp:hp + 64, :]
                        kT_h = qkT[6 + pr][hp:hp + 64, :]
                        for tsb, lhs_full, cdram, eng in (
                                (t1_sb, qT_h, c1d, nc.sync),
                                (t2_sb, kT_h, c2d, nc.scalar)):
                            stg = sb3.tile([128, CROW], fp16,
                                           tag="cstg", bufs=2)
                            for bt in range(NQT):
                                j0 = 384 - 128 * bt
                                pa = ps_mm.tile([128, 512], f32, tag="mm")
                                nc.tensor.matmul(
                                    pa[:],
                                    lhs_full[:, bt * 128:(bt + 1) * 128],
                                    tsb[hp:hp + 64, j0:j0 + 512],
                                    start=True, stop=True)
                                pb_t = ps_aux.tile([128, 128], f32,
                                                   tag="aux", bufs=2)
                                pb = pb_t[:, 0:128]
                                nc.tensor.matmul(
                                    pb,
                                    lhs_full[:, bt * 128:(bt + 1) * 128],
                                    tsb[hp:hp + 64, j0 + 512:j0 + WEXP],
                                    start=True, stop=True)
                                # balance the PSUM->SBUF staging copies
                                # across DVE and ACT (Pool cannot read PSUM)
                                if bt % 2 == 0:
                                    nc.vector.tensor_copy(
                                        stg[:, bt * WEXP:bt * WEXP + 512], pa[:])
                                    nc.scalar.copy(
                                        stg[:, bt * WEXP + 512:(bt + 1) * WEXP], pb)
                                else:
                                    nc.scalar.copy(
                                        stg[:, bt * WEXP:bt * WEXP + 512], pa[:])
                                    nc.vector.tensor_copy(
                                        stg[:, bt * WEXP + 512:(bt + 1) * WEXP], pb)
                            eng.dma_start(cdram[par, h, :, :], stg[:])
                        # merged skew reads (all 4 q/k tiles in one DMA each),
                        # same ring as the writes they depend on
                        cbase = (par * NH + h) * 128 * CROW
                        c2p_t = sb3.tile([128, 2048], fp16, tag="c2p",
                                         name="c2p", bufs=2)
                        nc.sync.dma_start(
                            c2p_t[:],
                            bass.AP(c1d, cbase + 127,
                                    [[CROW - 1, 128], [WEXP, 4], [1, 512]]))
                        c2p_tiles[h] = c2p_t
                        p2c_t = sb3.tile([128, 2048], fp16, tag="p2c",
                                         name="p2c", bufs=2)
                        nc.scalar.dma_start(
                            p2c_t[:],
                            bass.AP(c2d, cbase + 127,
                                    [[CROW - 1, 128], [WEXP, 4], [1, 512]]))
                        p2c_tiles[h] = p2c_t

                # ---- V projection: natural layout [tok, o]; runs while the
                # first heads' C blocks make their DRAM round trip ----
                v_sb = [sb.tile([128, H], bf16, tag=f"v{tt}", name=f"v{tt}", bufs=1)
                        for tt in range(NQT)]
                for lo, w in ((0, 512), (512, 256)):
                    for tt in range(NQT):
                        po = ps_mm.tile([128, 512], f32, tag="mm")
                        for hc in range(NHT):
                            nc.tensor.matmul(po[:, :w],
                                             hsT[hc][:, tt * 128:(tt + 1) * 128],
                                             wv[:, hc * 768 + lo:hc * 768 + lo + w],
                                             start=(hc == 0), stop=False,
                                             skip_group_check=True)
                        nc.tensor.matmul(po[:, :w], ones_row[:], bv[:, lo:lo + w],
                                         start=False, stop=True,
                                         skip_group_check=True)
                        nc.scalar.copy(v_sb[tt][:, lo:lo + w], po[:, :w])

                # ---- attention, per head-pair; ctx kept transposed ----
                wo = wpool.tile([128, 4608], fp16, tag="wout", bufs=1)
                nc.scalar.dma_start(wo[:], wout[li, :, :])
                ctxg = []       # 6 x [128, 512] fp16: ctx^T / den
                for pr in range(NH // 2):
                    ctxP_ps = ps_ctx.tile([128, 512], f32, tag="ctxT", bufs=1)
                    recb_ps = ps_ctx.tile([128, 512], f32, tag="recb")
                    for sub in range(2):
                        h = 2 * pr + sub
                        hp = sub * 64
                        tpos = (0, hp) if hp else None
                        qT_h = qkT[pr][hp:hp + 64, :]
                        kT_h = qkT[6 + pr][hp:hp + 64, :]
                        c2p_t = c2p_tiles[h]
                        p2c_t = p2c_tiles[h]
                        # scores: c2c + positional terms accumulate in PSUM;
                        # mask goes in as the Exp bias.
                        den_t = ps_ctx.tile([128, 512], f32, tag="recb")
                        den_ps = den_t[0:1, :]
                        for kt in range(NQT):
                            ps_s = ps_mm.tile([128, 512], f32, tag="mm")
                            nc.tensor.matmul(ps_s[:],
                                             kT_h[:, kt * 128:(kt + 1) * 128],
                                             qT_h[:], start=True, stop=True)
                            for qt in range(NQT):
                                nc.tensor.matmul(
                                    ps_s[:, qt * 128:(qt + 1) * 128],
                                    c2p_t[:, qt * 512 + kt * 128:qt * 512 + (kt + 1) * 128],
                                    ident[:], start=False, stop=True,
                                    skip_group_check=True)
                            nc.tensor.matmul(ps_s[:], ident[:],
                                             p2c_t[:, kt * 512:(kt + 1) * 512],
                                             start=False, stop=True,
                                             skip_group_check=True)
                            pT = sb3.tile([128, 512], bf16, tag="pT", name="pT")
                            nc.scalar.activation(pT[:], ps_s[:], EXPF,
                                                 bias=maskb[:, kt:kt + 1])
                            nc.tensor.matmul(den_ps, ones_col[:], pT[:],
                                             start=(kt == 0), stop=(kt == NQT - 1),
                                             skip_group_check=True)
                            nc.tensor.matmul(ctxP_ps[hp:hp + 64, :],
                                             v_sb[kt][:, h * 64:(h + 1) * 64],
                                             pT[:],
                                             start=(kt == 0), stop=(kt == NQT - 1),
                                             skip_group_check=True,
                                             tile_position=tpos)
                        rec = sb.tile([1, 512], f32, tag="rec")
                        nc.vector.reciprocal_approx_fast(rec[:], den_ps)
                        rech = sb.tile([1, 512], fp16, tag="rech")
                        nc.vector.tensor_copy(rech[:], rec[:])
                        # fp16 broadcast matmul (4x cheaper than f32 rows)
                        nc.tensor.matmul(recb_ps[hp:hp + 64, :],
                                         ones_row[:, 0:64], rech[:],
                                         start=True, stop=True,
                                         skip_group_check=True,
                                         tile_position=tpos)
                    # cg = ctx^T * gelu(g) * (1/den), all in [feature, token]
                    # (two TTs: each may read only one PSUM operand)
                    cx = sb.tile([128, 512], fp16, tag="cx", bufs=2)
                    nc.vector.tensor_tensor(cx[:], ctxP_ps[:], gT[pr][:],
                                            op=MULT)
                    cg = sb.tile([128, 512], fp16, tag=f"cg{pr}",
                                 name=f"cg{pr}", bufs=1)
                    nc.vector.tensor_tensor(cg[:], cx[:], recb_ps[:], op=MULT)
                    ctxg.append(cg)

                # ---- cg LN stats (partition-axis, via ones-matmuls) ----
                cgT = ctxg      # 6 x [128, 512] fp16: ctx^T * gelu(g) / den
                sum_t = ps_ctx.tile([128, 512], f32, tag="recb")
                sqs_t = ps_ctx.tile([128, 512], f32, tag="recb")
                sum_ps, sqs_ps = sum_t[0:1, :], sqs_t[0:1, :]
                for pr in range(NH // 2):
                    cg = cgT[pr]
                    sq = sb.tile([128, 512], fp16, tag=f"gT{pr}",
                                 name=f"sq{pr}", bufs=1)
                    nc.gpsimd.tensor_tensor(sq[:], cg[:], cg[:], op=MULT)
                    nc.tensor.matmul(sum_ps, ones_col16[:], cg[:],
                                     start=(pr == 0), stop=(pr == NHT - 1),
                                     skip_group_check=True)
                    nc.tensor.matmul(sqs_ps, ones_col16[:], sq[:],
                                     start=(pr == 0), stop=(pr == NHT - 1),
                                     skip_group_check=True)
                rmean_c, negrstd_c = stats_to_cols(sum_ps, sqs_ps, H, "cgs")

                # broadcast rows: ones (x) Woutsum, ones (x) bout
                wsum_sb = wpool.tile([1, 1536], fp16, tag="wsum", bufs=2)
                nc.sync.dma_start(wsum_sb[:], wsumd[li, :, :])
                bo = wpool.tile([1, H], fp16, tag="bout", bufs=2)
                nc.sync.dma_start(bo[:], boutd[li, :, :])
                wob_sb = sb.tile([128, H], fp16, tag="wob", bufs=1)
                bb_sb = sb.tile([128, H], fp16, tag="bb", bufs=1)
                for lo, w in ((0, 512), (512, 256)):
                    pw = ps_mm.tile([128, 512], f32, tag="mm")
                    nc.tensor.matmul(pw[:, :w], ones_row[:], wsum_sb[:, lo:lo + w],
                                     start=True, stop=True, skip_group_check=True)
                    nc.scalar.copy(wob_sb[:, lo:lo + w], pw[:, :w])
                    pb2 = ps_mm.tile([128, 512], f32, tag="mm")
                    nc.tensor.matmul(pb2[:, :w], ones_row[:], bo[:, lo:lo + w],
                                     start=True, stop=True, skip_group_check=True)
                    nc.scalar.copy(bb_sb[:, lo:lo + w], pb2[:, :w])

                # ---- out proj from cgT with LN folded into the epilogue ----
                for qt in range(NQT):
                    nc.vector.tensor_add(x[qt][:], x[qt][:], bb_sb[:])
                    veng = nc.vector
                    for lo, w in ((0, 512), (512, 256)):
                        po = ps_mm.tile([128, 512], f32, tag="mm")
                        for hc in range(NHT):
                            nc.tensor.matmul(po[:, :w],
                                             cgT[hc][:, qt * 128:(qt + 1) * 128],
                                             wo[:, hc * 768 + lo:hc * 768 + lo + w],
                                             start=(hc == 0), stop=(hc == NHT - 1),
                                             skip_group_check=True)
                        pe = sb.tile([128, 512], fp16, tag="poev")
                        nc.scalar.copy(pe[:, :w], po[:, :w])
                        t = sb.tile([128, 512], f32, tag="fold")
                        veng.scalar_tensor_tensor(
                            t[:, :w], wob_sb[:, lo:lo + w], rmean_c[:, qt:qt + 1],
                            pe[:, :w], op0=MULT, op1=SUB)
                        veng.scalar_tensor_tensor(
                            x[qt][:, lo:lo + w], t[:, :w], negrstd_c[:, qt:qt + 1],
                            x[qt][:, lo:lo + w], op0=MULT, op1=ADD)

                # ---- FFN (transposed: W1 emits [feature, token]) ----
                h2 = ln_group(x, H, "hs")
                h2T = transpose_h(h2, NHT, "h2T")
                a_tiles = [sb.tile([128, 512], fp16, tag=f"tp{i}", name=f"a{i}",
                                   bufs=1) for i in range(16)]
                sum2_t = ps_ctx.tile([128, 512], f32, tag="recb")
                sqs2_t = ps_ctx.tile([128, 512], f32, tag="recb")
                sum2_ps, sqs2_ps = sum2_t[0:1, :], sqs2_t[0:1, :]
                # W2 halves loaded up front so the DMAs overlap the W1 phase
                w2h = []
                for hh in range(2):
                    wt2 = wpool.tile([128, 6144], fp16, tag=f"w2h{hh}", bufs=1)
                    eng = nc.sync if hh == 0 else nc.scalar
                    eng.dma_start(wt2[:], w2[li, hh, :, :])
                    w2h.append(wt2)
                for g8 in range(8):
                    # half-blob loads: finer-grained prefetch pipeline
                    wf_h = []
                    for wh in range(2):
                        t = wpool.tile([128, 1536], fp16, tag="w1", bufs=3)
                        eng = nc.sync if (2 * g8 + wh) % 2 == 0 else nc.scalar
                        eng.dma_start(
                            t[:], w1[li, g8, :, wh * 1536:(wh + 1) * 1536])
                        wf_h.append(t)
                    for otl in range(4):
                        ot = g8 * 4 + otl
                        wf = wf_h[otl // 2]
                        wlo = (otl % 2) * 768
                        po = ps_mm.tile([128, 512], f32, tag="mm")
                        for hc in range(NHT):
                            nc.tensor.matmul(
                                po[:],
                                wf[:, wlo + hc * 128:wlo + (hc + 1) * 128],
                                h2T[hc][:],
                                start=(hc == 0), stop=(hc == NHT - 1))
                        if ot < 16:
                            nc.scalar.copy(a_tiles[ot][:], po[:])
                        else:
                            # native gelu straight out of PSUM (table already
                            # on the gelu set since the gate gelus above)
                            at = a_tiles[ot - 16]
                            gt = sb.tile([128, 512], fp16, tag="ffng")
                            nc.scalar.activation(gt[:], po[:], GELU)
                            nc.vector.tensor_mul(at[:], at[:], gt[:])
                            # tile (ot-16) is final: accumulate its LN stats now
                            i = ot - 16
                            nc.tensor.matmul(sum2_ps, ones_col16[:], at[:],
                                             start=(i == 0), stop=(i == 15),
                                             skip_group_check=True)
                            sq = sb.tile([128, 512], fp16, tag="sq")
                            nc.gpsimd.tensor_tensor(sq[:], at[:], at[:], op=MULT)
                            nc.tensor.matmul(sqs2_ps, ones_col16[:], sq[:],
                                             start=(i == 0), stop=(i == 15),
                                             skip_group_check=True)

                rmean2, negrstd2 = stats_to_cols(sum2_ps, sqs2_ps, FI, "uns")
                w2b_sb = sb.tile([128, H], fp16, tag="w2b", bufs=1)
                for lo, w in ((0, 512), (512, 256)):
                    pw = ps_mm.tile([128, 512], f32, tag="mm")
                    nc.tensor.matmul(pw[:, :w], ones_row[:],
                                     wsum_sb[:, 768 + lo:768 + lo + w],
                                     start=True, stop=True, skip_group_check=True)
                    nc.scalar.copy(w2b_sb[:, lo:lo + w], pw[:, :w])

                # ---- W2 from raw GeGLU tiles with LN folded in ----
                for qt in range(NQT):
                    veng = nc.vector
                    for lo, w in ((0, 512), (512, 256)):
                        po = ps_mm.tile([128, 512], f32, tag="mm")
                        for ic in range(16):
                            nc.tensor.matmul(
                                po[:, :w], a_tiles[ic][:, qt * 128:(qt + 1) * 128],
                                w2h[ic // 8][:, (ic % 8) * 768 + lo:(ic % 8) * 768 + lo + w],
                                start=(ic == 0), stop=(ic == 15),
                                skip_group_check=True)
                        pe = sb.tile([128, 512], fp16, tag="poev")
                        nc.scalar.copy(pe[:, :w], po[:, :w])
                        t = sb.tile([128, 512], f32, tag="fold")
                        veng.scalar_tensor_tensor(
                            t[:, :w], w2b_sb[:, lo:lo + w], rmean2[:, qt:qt + 1],
                            pe[:, :w], op0=MULT, op1=SUB)
                        veng.scalar_tensor_tensor(
                            x[qt][:, lo:lo + w], t[:, :w], negrstd2[:, qt:qt + 1],
                            x[qt][:, lo:lo + w], op0=MULT, op1=ADD)

            # ---- output ----
            for qt in range(NQT):
                nc.sync.dma_start(yd[qt, :, :], x[qt][:])

    nc.finalize()
    return nc


_CACHE = {}


def _get_nc(n_layers, passes=1):
    key = (n_layers, passes)
    if key not in _CACHE:
        _CACHE[key] = _build(n_layers, passes)
    return _CACHE[key]


# ---------------------------------------------------------------- host prep
def _prep_shared(word_emb, rel_emb, rel_g, rel_b, Wqk, bqk, Wvg, bvg, Wout,
                 bout, W1, W2, n_layers):
    beta = _beta_delta()                     # [1023]
    idx_c2p = beta[1022 - np.arange(1023)]   # T1: delta = 511 - j
    idx_p2c = beta[np.arange(1023)]          # T2: delta = j - 511
    rel = _ln_np(rel_emb.astype(np.float64)).astype(np.float32) * rel_g + rel_b

    d = {}
    t1 = np.zeros((n_layers, NH, 64, 1024), np.float32)  # packed to pairs below
    t2 = np.zeros((n_layers, NH, 64, 1024), np.float32)
    wqk_t = np.zeros((n_layers, 18, 128, 768), np.float32)  # regrouped below
    wvg_t = np.zeros((n_layers, 128, 4608), np.float32)
    wout_t = np.zeros((n_layers, 128, 4608), np.float32)
    w1_t = np.zeros((n_layers, 8, 128, 3072), np.float32)
    w2_t = np.zeros((n_layers, 2, 128, 6144), np.float32)
    bqk_t = np.zeros((n_layers, 128, 12), np.float32)
    bg_t = np.zeros((n_layers, 128, 6), np.float32)
    bvg_t = np.zeros((n_layers, 1, H), np.float32)
    bout_t = np.zeros((n_layers, 1, H), np.float32)
    wsum_t = np.zeros((n_layers, 1, 1536), np.float32)
    for li in range(n_layers):
        pos = rel @ Wqk[li].T + bqk[li]          # [63, 1536]
        qpos = pos[:, :H].reshape(63, NH, 64)
        kpos = pos[:, H:].reshape(63, NH, 64)
        # T1[j] = kpos[beta(511-j)], T2[j] = qpos[beta(j-511)] * SCALE
        t1[li, :, :, :1023] = kpos[idx_c2p].transpose(1, 2, 0)
        t2[li, :, :, :1023] = qpos[idx_p2c].transpose(1, 2, 0) * SCALE

        wqkT = Wqk[li].T.copy()                  # [768, 1536]
        # qk blobs: [ot, p, hc*128+oo] = wqkT[hc*128+p, ot*128+oo]
        wqk_t[li, :12] = (wqkT.reshape(NHT, 128, 12, 128)
                          .transpose(2, 1, 0, 3).reshape(12, 128, 768))
        wvgT = Wvg[li].T.copy()                  # [768, 1536]
        # gate blobs (transposed proj): [got, p, hc*128+oo]
        wqk_t[li, 12:] = (wvgT[:, H:].reshape(NHT, 128, NHT, 128)
                          .transpose(2, 1, 0, 3).reshape(6, 128, 768))
        # V rhs blob: [p, hc*768+c] = wvgT[hc*128+p, c]
        wvg_t[li] = (wvgT[:, :H].reshape(NHT, 128, H)
                     .transpose(1, 0, 2).reshape(128, 4608))
        wout_t[li] = (Wout[li].T.reshape(NHT, 128, H)
                      .transpose(1, 0, 2).reshape(128, 4608))
        # W1 lhsT blobs (transposed proj): [g, p, otl*768 + hc*128 + oo]
        w1_t[li] = (W1[li].T.reshape(NHT, 128, 32, 128)
                    .transpose(2, 1, 0, 3).reshape(8, 4, 128, NHT * 128)
                    .transpose(0, 2, 1, 3).reshape(8, 128, 3072))
        w2_t[li] = (W2[li].T.reshape(2, 8, 128, H)
                    .transpose(0, 2, 1, 3).reshape(2, 128, 6144))
        bqk_t[li] = bqk[li].reshape(12, 128).T
        bg_t[li] = bvg[li][H:].reshape(6, 128).T
        bvg_t[li, 0] = bvg[li][:H]
        bout_t[li, 0] = bout[li]
        wsum_t[li, 0, :H] = Wout[li].sum(axis=1)
        wsum_t[li, 0, H:] = W2[li].sum(axis=1)

    # regroup: [li, pr, :, 0:768|768:1536|1536:2304] = ots (pr, 6+pr, 12+pr)
    wqk_g = np.concatenate([wqk_t[:, 0:6], wqk_t[:, 6:12], wqk_t[:, 12:18]],
                           axis=3)
    d["wqk"] = wqk_g.astype(F16)
    d["wvg"] = wvg_t.astype(F16)
    d["wout"] = wout_t.astype(F16)
    d["w1"] = w1_t.astype(F16)
    d["w2"] = w2_t.astype(F16)
    d["t12d"] = np.concatenate(
        [t1.reshape(n_layers, NH // 2, 128, 1024),
         t2.reshape(n_layers, NH // 2, 128, 1024)], axis=3).astype(F16)
    d["bqkd"] = bqk_t
    d["bgd"] = bg_t
    d["bvgd"] = bvg_t.astype(F16)
    d["boutd"] = bout_t.astype(F16)
    d["wsumd"] = wsum_t.astype(F16)
    return d


def _make_in_maps(inputs, n_layers):
    input_ids = np.asarray(inputs["input_ids"])
    attention_mask = np.asarray(inputs["attention_mask"])
    word_emb = np.asarray(inputs["word_emb"], np.float32)

    shared = _prep_shared(
        word_emb, np.asarray(inputs["rel_emb"], np.float32),
        np.asarray(inputs["rel_g"], np.float32), np.asarray(inputs["rel_b"], np.float32),
        np.asarray(inputs["Wqk"], np.float32), np.asarray(inputs["bqk"], np.float32),
        np.asarray(inputs["Wvg"], np.float32), np.asarray(inputs["bvg"], np.float32),
        np.asarray(inputs["Wout"], np.float32), np.asarray(inputs["bout"], np.float32),
        np.asarray(inputs["W1"], np.float32), np.asarray(inputs["W2"], np.float32),
        n_layers)

    in_maps = []
    for b in range(B):
        m = dict(shared)
        x0 = _ln_np(word_emb[input_ids[:, b]].astype(np.float64)).astype(np.float32)
        m["x0d"] = x0.reshape(NQT, 128, H)
        mb = np.where(attention_mask[b, 0, 0, :], MASK_NEG, 0.0).astype(np.float32)
        m["maskd"] = mb.reshape(NQT, 128).T.copy()
        in_maps.append(m)
    return in_maps


def run(inputs, n_layers=L, trace=False):
    nc = _get_nc(n_layers)
    in_maps = _make_in_maps(inputs, n_layers)
    res = run_bass_kernel_spmd(nc, in_maps, core_ids=list(range(B)), trace=trace)
    out = np.zeros((S, B, H), np.float32)
    for b in range(B):
        out[:, b, :] = res.results[b]["yd"].reshape(S, H)
    return out, res


def kernel(**inputs) -> np.ndarray:
    out, _ = run(inputs, L)
    return out


# ------------------------------------------------------- timing-only runner
def make_timed_runner(n_layers, in_maps, passes=1):
    """Build a persistent jitted PJRT callable over 8 cores for wall-clock
    timing (the axon NTFF profile hook is unavailable in this container).
    passes>1 builds the NEFF that repeats the full forward back-to-back;
    timing two pass counts and taking the slope cancels the fixed axon/PJRT
    dispatch overhead, leaving the true per-execution HW time."""
    import jax
    from jax.sharding import Mesh, PartitionSpec, NamedSharding
    from jax.experimental.shard_map import shard_map
    from concourse import bass2jax

    nc = _get_nc(n_layers, passes)
    bass2jax.install_neuronx_cc_hook()
    partition_name = nc.partition_id_tensor.name if nc.partition_id_tensor else None
    in_names, out_names, out_avals, zero_outs = [], [], [], []
    import concourse.mybir as _mb
    for alloc in nc.m.functions[0].allocations:
        if not isinstance(alloc, _mb.MemoryLocationSet):
            continue
        name = alloc.memorylocations[0].name
        if alloc.kind == "ExternalInput":
            if name != partition_name:
                in_names.append(name)
        elif alloc.kind == "ExternalOutput":
            out_names.append(name)
            shape = tuple(alloc.tensor_shape)
            dtype = _mb.dt.np(alloc.dtype)
            out_avals.append(jax.core.ShapedArray(shape, dtype))
            zero_outs.append(np.zeros(shape, dtype))
    n_params = len(in_names)
    n_outs = len(out_avals)
    all_in_names = list(in_names) + out_names
    if partition_name is not None:
        all_in_names = all_in_names + [partition_name]

    def _body(*args):
        operands = list(args)
        if partition_name is not None:
            operands.append(bass2jax.partition_id_tensor())
        outs = bass2jax._bass_exec_p.bind(
            *operands, out_avals=tuple(out_avals), in_names=tuple(all_in_names),
            out_names=tuple(out_names), lowering_input_output_aliases=(),
            sim_require_finite=True, sim_require_nnan=True, nc=nc)
        return tuple(outs)

    n_cores = B
    devices = jax.devices()[:n_cores]
    mesh = Mesh(np.asarray(devices), ("core",))
    P = PartitionSpec
    sharded = jax.jit(
        shard_map(_body, mesh=mesh, in_specs=(P("core"),) * (n_params + n_outs),
                  out_specs=(P("core"),) * n_outs, check_rep=False),
        keep_unused=True)

    concat_in = [
        np.concatenate([np.asarray(in_maps[c][nm]) for c in range(n_cores)], axis=0)
        for nm in in_names]
    concat_zeros = [np.zeros((n_cores * z.shape[0], *z.shape[1:]), z.dtype)
                    for z in zero_outs]
    shard = NamedSharding(mesh, P("core"))
    dev_in = [jax.device_put(a, shard) for a in concat_in]
    dev_zeros = [jax.device_put(a, shard) for a in concat_zeros]

    def call():
        outs = sharded(*dev_in, *dev_zeros)
        jax.block_until_ready(outs)
        return outs

    return call
